# revision 31
# baseline (speedup 1.0000x reference)
"""Trainium2 Bass kernel for nn_GATv2Base (gnn_message_passing).

Contract: kernel(**inputs) takes FULL unsharded inputs (same keys as
reference.setup_inputs()) and returns the FULL [32, 64] float32 output.

Sharding: 32 graphs -> 8 cores (4 graphs each, contiguous node ranges since
`batch` is sorted).  Edges (plus self-loops) are routed to the core owning
their dst node, sorted by dst, and packed into "spans" (<=127-node dst window,
2304 edge slots = 18 subgroups of 128 edges).  Node features live in a
span-major padded global layout so every per-span device address is static.
Layer 1 runs fully local (xl1 table computed replicated from x); between
layers one fp16 AllGather shares the xl2 table; the pooled per-graph MLP is
computed on the owning core.  Only the [4, 64] per-core outputs return to the
host.

All per-core inputs are packed into 5 device tensors (feat/wpack/meta_idx/
meta_val/mlp) to minimize per-call argument-marshalling overhead on the host
runtime.  _build_program(reps=N) unrolls the whole computation N times inside
one program; timing two variants isolates true device execution time from
dispatch overhead.
"""

import os
import sys

import numpy as np

for _p in ("/opt/trn_rl_repo", "/root/.axon_site/_ro/trn_rl_repo"):
    if os.path.isdir(_p) and _p not in sys.path:
        sys.path.insert(0, _p)

import concourse.bass as bass
import concourse.bacc as bacc
import concourse.mybir as mybir
import concourse.tile as tile
from concourse.bass_utils import run_bass_kernel_spmd

F32 = mybir.dt.float32
F16 = mybir.dt.float16
F8 = mybir.dt.float8e4
I16 = mybir.dt.int16
I32 = mybir.dt.int32
AF = mybir.ActivationFunctionType
ALU = mybir.AluOpType
AXX = mybir.AxisListType.X

N, E, H, C, NG = 50000, 800000, 4, 64, 32
HC = H * C
NCORES = 8
SLOT_H = 1152            # edge slots per table-half region (9 subgroups)
SPAN_EDGES = 2 * SLOT_H  # 2304 edge slots per span (18 subgroups of 128)
NSG = SPAN_EDGES // 128  # 18
NSG_H = SLOT_H // 128    # 9
SPAN_DST = 127           # dst window per span; slot 127 = pad marker

# meta_idx layout (i16 cols): src_idx_a 0:72, src_idx_b 72:144, xr_idx 144:288
MI_A0, MI_A1 = 0, SLOT_H // 16
MI_B0, MI_B1 = MI_A1, 2 * (SLOT_H // 16)
MI_X0, MI_X1 = MI_B1, MI_B1 + SPAN_EDGES // 16
# meta_val layout (f16 cols): dcol 0:18, eac 18:36, gmask 36:40
MV_D0, MV_D1 = 0, NSG
MV_E0, MV_E1 = NSG, 2 * NSG
MV_G0, MV_G1 = 2 * NSG, 2 * NSG + 4
MV_W = MV_G1
# wpack rows (f16, width 256)
WP_WL1, WP_WR1 = 0, 65
WP_ATT1, WP_WE1, WP_BIAS1 = 130, 258, 386
WP_WL2, WP_WR2 = 514, 771
WP_ATT2, WP_WE2, WP_BIAS2 = 1028, 1156, 1284
WP_IOTA = 1412
WP_ROWS = 1540
# mlp rows (f32, width 128)
MP_P1, MP_LNG, MP_LNB, MP_P2, MP_CNT = 0, 257, 261, 265, 394
MP_ROWS = 398


# ----------------------------------------------------------------------------
# Host-side sharding / packing
# ----------------------------------------------------------------------------

def _host_prep(inp):
    x = np.asarray(inp["x"], dtype=np.float32)
    ei = np.asarray(inp["edge_index"], dtype=np.int32)
    ea_full = np.asarray(inp["edge_attr"], dtype=np.float32)[:, 0]
    batch = np.asarray(inp["batch"], dtype=np.int32)

    src0, dst0 = ei[0], ei[1]
    deg = np.maximum(np.bincount(dst0, minlength=N).astype(np.float64), 1.0)
    loop_attr = (
        np.bincount(dst0, weights=ea_full.astype(np.float64), minlength=N) / deg
    ).astype(np.float32)
    src = np.concatenate([src0, np.arange(N, dtype=np.int32)])
    dst = np.concatenate([dst0, np.arange(N, dtype=np.int32)])
    eattr = np.concatenate([ea_full, loop_attr]).astype(np.float32)

    gcounts = np.bincount(batch, minlength=NG)
    gstart = np.concatenate([[0], np.cumsum(gcounts)])
    core_n0 = np.array([gstart[4 * k] for k in range(NCORES)] + [N], dtype=np.int64)

    order = np.argsort(dst, kind="stable")
    src, dst, eattr = src[order], dst[order], eattr[order]
    edge_lo = np.searchsorted(dst, core_n0[:-1], "left")
    edge_hi = np.searchsorted(dst, core_n0[1:], "left")

    # src owner core (cores 0-3 -> table half A, 4-7 -> half B); stable
    # under span-count changes so it can drive packing.
    src_owner = np.searchsorted(core_n0[1:], src, "right")
    src_in_a = src_owner < (NCORES // 2)

    cores = []
    for k in range(NCORES):
        n0, n1 = int(core_n0[k]), int(core_n0[k + 1])
        s, e = int(edge_lo[k]), int(edge_hi[k])
        cd = dst[s:e]
        ca = src_in_a[s:e]
        nlocal = n1 - n0
        node_edge_start = np.searchsorted(cd, n0 + np.arange(nlocal + 1))
        cumA = np.concatenate([[0], np.cumsum(ca)])  # over edges
        spans = []
        b = 0
        while b < nlocal:
            bend = b
            while bend < nlocal and (bend - b) < SPAN_DST:
                e0, e1 = node_edge_start[b], node_edge_start[bend + 1]
                nA = cumA[e1] - cumA[e0]
                nB = (e1 - e0) - nA
                if nA > SLOT_H or nB > SLOT_H:
                    break
                bend += 1
            assert bend > b, "single node exceeds span edge capacity"
            spans.append(
                (b, bend - b, int(node_edge_start[b]), int(node_edge_start[bend]))
            )
            b = bend
        cores.append(
            dict(n0=n0, n1=n1, spans=spans, src=src[s:e], dst=cd, ea=eattr[s:e],
                 in_a=ca)
        )

    nspans = max(len(c["spans"]) for c in cores)
    rows_per_core = nspans * 128
    rows_total = NCORES * rows_per_core

    # global padded row per node
    node_row = np.zeros(N, dtype=np.int64)
    for k, c in enumerate(cores):
        for si, (b, nb, _, _) in enumerate(c["spans"]):
            nodes = np.arange(c["n0"] + b, c["n0"] + b + nb)
            node_row[nodes] = k * rows_per_core + si * 128 + (nodes - c["n0"] - b)

    # x padded, transposed, with ones row (for encoder rhs)
    x_pad = np.zeros((rows_total, 4), dtype=np.float32)
    x_pad[node_row] = x
    x_aug_T = np.concatenate(
        [x_pad.T, np.ones((1, rows_total), dtype=np.float32)], axis=0
    )  # [5, R]

    packs = []
    half_rows = rows_total // 2
    assert half_rows <= 32767, f"table half {half_rows} exceeds int16 index range"

    def wrap_idx16(vals):
        # vals: [SLOT] int -> wrapped [128, SLOT//16] int16 (16-part wrap,
        # replicated over the 8 q7 core groups)
        slot = len(vals)
        base = np.zeros((16, slot // 16), dtype=np.int16)
        i = np.arange(slot)
        base[i % 16, i // 16] = vals.astype(np.int16)
        return np.tile(base, (8, 1))

    for k, c in enumerate(cores):
        meta_idx = np.zeros((nspans, 128, MI_X1), dtype=np.int16)
        meta_val = np.zeros((nspans, 128, MV_W), dtype=np.float16)
        ea_T = np.zeros((nspans, SPAN_EDGES), dtype=np.float16)
        meta_val[:, :, MV_D0:MV_D1] = np.float16(127.0)
        for si, (b, nb, e0, e1) in enumerate(c["spans"]):
            ina = c["in_a"][e0:e1]
            esrc = node_row[c["src"][e0:e1]]
            edrel = (c["dst"][e0:e1] - c["n0"] - b).astype(np.int64)
            eea = c["ea"][e0:e1]
            # slots: A edges first (in region [0, SLOT_H)), then B edges at
            # [SLOT_H, 2*SLOT_H); pads keep idx 0 / drel 127 / ea 0
            ia = np.where(ina)[0]
            ib = np.where(~ina)[0]
            slots = np.empty(len(ina), dtype=np.int64)
            slots[ia] = np.arange(len(ia))
            slots[ib] = SLOT_H + np.arange(len(ib))
            av = np.zeros(SLOT_H, dtype=np.int64)
            av[:len(ia)] = esrc[ia]
            bv = np.zeros(SLOT_H, dtype=np.int64)
            bv[:len(ib)] = esrc[ib] - half_rows
            meta_idx[si, :, MI_A0:MI_A1] = wrap_idx16(av)
            meta_idx[si, :, MI_B0:MI_B1] = wrap_idx16(bv)
            xv = np.full(SPAN_EDGES, si * 128 + 127, dtype=np.int64)
            xv[slots] = si * 128 + edrel
            meta_idx[si, :, MI_X0:MI_X1] = wrap_idx16(xv)
            p, sg = slots % 128, slots // 128
            meta_val[si, p, MV_D0 + sg] = edrel.astype(np.float16)
            meta_val[si, p, MV_E0 + sg] = eea.astype(np.float16)
            ea_T[si, slots] = eea.astype(np.float16)
            nodes = np.arange(c["n0"] + b, c["n0"] + b + nb)
            gl = batch[nodes] - 4 * k
            meta_val[si, np.arange(nb), MV_G0 + gl] = np.float16(1.0)
        inv_cnt = np.zeros((4,), dtype=np.float32)
        for gg in range(4):
            cnt = max(int(gcounts[4 * k + gg]), 1)
            inv_cnt[gg] = 1.0 / cnt
        packs.append(
            dict(
                meta_idx=meta_idx,
                meta_val=meta_val,
                ea_T=ea_T,
                inv_cnt=inv_cnt,
                own_cols=np.arange(
                    k * rows_per_core, (k + 1) * rows_per_core, dtype=np.int64
                ),
            )
        )
    return cores, packs, nspans, rows_per_core, rows_total, x_aug_T, node_row


# ----------------------------------------------------------------------------
# Device program
# ----------------------------------------------------------------------------

_PROGRAM_CACHE = {}


def _build_program(nspans, rows_total, reps=1, phase_limit=5, op_limit=9,
                   single_packet=False, nqueues=4, skip_r=False,
                   debug_h1=False):
    rows_per_core = nspans * 128
    nblocks = rows_total // 128

    nc = bacc.Bacc(num_swdge_queues=nqueues)
    tcx = tile.TileContext(nc)

    t_feat = nc.dram_tensor(
        "feat", [5, rows_total + rows_per_core + 64], F32, kind="ExternalInput"
    )
    t_wpack = nc.dram_tensor("wpack", [WP_ROWS, HC], F16, kind="ExternalInput")
    t_midx = nc.dram_tensor(
        "meta_idx", [nspans, 128, MI_X1], I16, kind="ExternalInput"
    )
    t_mval = nc.dram_tensor(
        "meta_val", [nspans, 128, MV_W], F16, kind="ExternalInput"
    )
    t_eaT = nc.dram_tensor("eaT", [nspans, SPAN_EDGES], F16, kind="ExternalInput")
    t_mlp = nc.dram_tensor("mlp", [MP_ROWS, 128], F32, kind="ExternalInput")
    t_out = nc.dram_tensor("out", [4, 64], F32, kind="ExternalOutput")

    # ---- internal DRAM ----
    t_xl1 = nc.dram_tensor("xl1_tbl", [rows_total, HC], F8)
    t_xr1 = nc.dram_tensor("xr1_own", [rows_per_core, HC], F8)
    t_h1 = nc.dram_tensor("h1_own", [rows_per_core, HC], F16)
    t_h1o = (nc.dram_tensor("h1dump", [rows_per_core, HC], F16,
                            kind="ExternalOutput") if debug_h1 else None)
    t_udump = (nc.dram_tensor("udump", [128, NSG * HC], F16,
                              kind="ExternalOutput") if debug_h1 else None)
    t_gdump = (nc.dram_tensor("gdump", [128, NSG * HC], F16,
                              kind="ExternalOutput") if debug_h1 else None)
    t_adump = (nc.dram_tensor("adump", [128, 4 * NSG], F32,
                              kind="ExternalOutput") if debug_h1 else None)
    t_edump = (nc.dram_tensor("edump", [128, NSG * HC], F16,
                              kind="ExternalOutput") if debug_h1 else None)
    t_acdump = (nc.dram_tensor("acdump", [128, HC + 4], F32,
                               kind="ExternalOutput") if debug_h1 else None)
    t_mdump = (nc.dram_tensor("mdump", [128, NSG * HC], F16,
                              kind="ExternalOutput") if debug_h1 else None)
    t_xr2 = nc.dram_tensor("xr2_own", [rows_per_core, HC], F8)
    t_xl2_in = nc.dram_tensor("xl2_own_cc", [rows_per_core, HC], F8)
    t_xl2 = nc.dram_tensor("xl2_tbl", [rows_total, HC], F8, addr_space="Shared")

    from contextlib import ExitStack
    with tcx as tc, ExitStack() as es:
        # ------------------------------------------------------------------
        # constants in SBUF (loaded once, reused by every rep)
        # ------------------------------------------------------------------
        cpool = es.enter_context(tc.tile_pool(name="consts", bufs=1))
        enc_aug = cpool.tile([5, 64], F32)
        nc.sync.dma_start(
            out=enc_aug[:],
            in_=t_feat[:, rows_total + rows_per_core:rows_total + rows_per_core + 64],
        )
        iota_rep = cpool.tile([128, 128], F16)
        nc.sync.dma_start(out=iota_rep[:], in_=t_wpack[WP_IOTA:WP_IOTA + 128, 0:128])
        reps_t = {}
        for L, (r_att, r_we, r_bias) in (
            (1, (WP_ATT1, WP_WE1, WP_BIAS1)),
            (2, (WP_ATT2, WP_WE2, WP_BIAS2)),
        ):
            for nm, r0 in (("att_row", r_att), ("we_row", r_we), ("bias_row", r_bias)):
                rep = cpool.tile([128, HC], F16, tag=f"rep{L}{nm}")
                nc.sync.dma_start(out=rep[:], in_=t_wpack[r0:r0 + 128, :])
                reps_t[(L, nm)] = rep
        ones_col = cpool.tile([1, 128], F16)
        nc.vector.memset(ones_col[:], 1.0)
        ones_row = cpool.tile([1, 512], F16)
        nc.vector.memset(ones_row[:], 1.0)
        from concourse.masks import make_identity
        ident16 = cpool.tile([128, 128], F16)
        make_identity(nc, ident16[:])
        ident8 = cpool.tile([128, 128], F8)
        nc.vector.tensor_copy(out=ident8[:], in_=ident16[:])

        wpool = es.enter_context(tc.tile_pool(name="weights", bufs=1))
        wl1 = wpool.tile([65, HC], F16)
        wr1 = wpool.tile([65, HC], F16)
        nc.sync.dma_start(out=wl1[:], in_=t_wpack[WP_WL1:WP_WL1 + 65, :])
        nc.sync.dma_start(out=wr1[:], in_=t_wpack[WP_WR1:WP_WR1 + 65, :])
        w2_tiles = {}
        for nm, r0 in (("wl_aug", WP_WL2), ("wr_aug", WP_WR2)):
            a = wpool.tile([128, HC], F16, tag=f"{nm}a")
            b = wpool.tile([128, HC], F16, tag=f"{nm}b")
            cbias = wpool.tile([1, HC], F16, tag=f"{nm}c")
            nc.sync.dma_start(out=a[:], in_=t_wpack[r0:r0 + 128, :])
            nc.sync.dma_start(out=b[:], in_=t_wpack[r0 + 128:r0 + 256, :])
            nc.sync.dma_start(out=cbias[:], in_=t_wpack[r0 + 256:r0 + 257, :])
            w2_tiles[nm] = (a, b, cbias)
        # MLP constants
        mpool = es.enter_context(tc.tile_pool(name="mlpc", bufs=1))
        p1a = mpool.tile([128, 128], F32)
        p1b = mpool.tile([128, 128], F32)
        p1c = mpool.tile([1, 128], F32)
        nc.sync.dma_start(out=p1a[:], in_=t_mlp[MP_P1:MP_P1 + 128, :])
        nc.sync.dma_start(out=p1b[:], in_=t_mlp[MP_P1 + 128:MP_P1 + 256, :])
        nc.sync.dma_start(out=p1c[:], in_=t_mlp[MP_P1 + 256:MP_P1 + 257, :])
        p2a = mpool.tile([128, 64], F32)
        p2c = mpool.tile([1, 64], F32)
        nc.sync.dma_start(out=p2a[:], in_=t_mlp[MP_P2:MP_P2 + 128, 0:64])
        nc.sync.dma_start(out=p2c[:], in_=t_mlp[MP_P2 + 128:MP_P2 + 129, 0:64])
        lng = mpool.tile([4, 128], F32)
        nc.sync.dma_start(out=lng[:], in_=t_mlp[MP_LNG:MP_LNG + 4, :])
        lnb = mpool.tile([4, 128], F32)
        nc.sync.dma_start(out=lnb[:], in_=t_mlp[MP_LNB:MP_LNB + 4, :])
        icnt = mpool.tile([4, 1], F32)
        nc.sync.dma_start(out=icnt[:], in_=t_mlp[MP_CNT:MP_CNT + 4, 0:1])
        ident = mpool.tile([128, 128], F32)
        from concourse.masks import make_identity
        make_identity(nc, ident[:])
        onesg = mpool.tile([1, 4], F32)
        nc.vector.memset(onesg[:], 1.0)

        def encode4(pool, ppool, col0, ncols):
            """Encode ncols (<=512) padded nodes starting at feat col col0.
            Returns h0T4 [65, ncols] f16 (aug ones row included)."""
            xT = pool.tile([5, 512], F32, tag="xT")
            nc.sync.dma_start(out=xT[:, 0:ncols], in_=t_feat[:, col0:col0 + ncols])
            h0p = ppool.tile([64, 512], F32, tag="h0ps")
            nc.tensor.matmul(out=h0p[:, 0:ncols], lhsT=enc_aug[:],
                             rhs=xT[:, 0:ncols], start=True, stop=True)
            h0T = pool.tile([65, 512], F16, tag="h0T")
            nc.scalar.activation(out=h0T[0:64, 0:ncols], in_=h0p[:, 0:ncols],
                                 func=AF.Relu)
            nc.vector.tensor_copy(out=h0T[64:65, 0:ncols],
                                  in_=ones_row[:, 0:ncols])
            return h0T

        def xw_blocks(pool, ppool, h0T, w, nblk, sink_ap, dt=F16):
            """nblk xl/xr matmuls from h0T slices; one batched DMA to DRAM."""
            xls = pool.tile([128, 4, HC], dt, tag=f"xls{mybir.dt.size(dt)}")
            for j in range(nblk):
                xlp = ppool.tile([128, HC], F32, tag="xlps")
                nc.tensor.matmul(out=xlp[:], lhsT=h0T[:, j * 128:(j + 1) * 128],
                                 rhs=w[:], start=True, stop=True)
                if j % 2 == 0:
                    nc.vector.tensor_copy(out=xls[:, j, :], in_=xlp[:])
                else:
                    nc.scalar.copy(out=xls[:, j, :], in_=xlp[:])
            nc.sync.dma_start(
                out=sink_ap.rearrange("(b p) c -> p b c", p=128),
                in_=xls[:, 0:nblk, :],
            )

        def build_rep():
            # --------------------------------------------------------------
            # Phase 1: encoder + xl1 for ALL rows (4 blocks per DMA batch)
            # --------------------------------------------------------------
            with tc.tile_pool(name="p1", bufs=3) as pool, \
                 tc.tile_pool(name="p1ps", bufs=2, space="PSUM") as ppool:
                nb4 = (nblocks + 3) // 4 if phase_limit >= 1 else 0
                for b4 in range(nb4):
                    nblk = min(4, nblocks - b4 * 4)
                    h0T = encode4(pool, ppool, b4 * 512, nblk * 128)
                    xw_blocks(pool, ppool, h0T, wl1, nblk,
                              t_xl1[b4 * 512:b4 * 512 + nblk * 128, :], dt=F8)
                ns4 = (nspans + 3) // 4 if phase_limit >= 1 else 0
                for s4 in range(ns4):
                    nblk = min(4, nspans - s4 * 4)
                    h0T = encode4(pool, ppool, rows_total + s4 * 512, nblk * 128)
                    xw_blocks(pool, ppool, h0T, wr1, nblk,
                              t_xr1[s4 * 512:s4 * 512 + nblk * 128, :], dt=F8)

            # --------------------------------------------------------------
            # GAT span loop (shared for both layers)
            # --------------------------------------------------------------
            def gat_layer(L, xl_tbl, xr_tbl, h_sink):
                """h_sink(s, htile, mval): consume flush output [128, HC] f16."""
                att_rep = reps_t[(L, "att_row")]
                we_rep = reps_t[(L, "we_row")]
                bias_rep = reps_t[(L, "bias_row")]
                with tc.tile_pool(name=f"g{L}", bufs=2) as pool, \
                     tc.tile_pool(name=f"g{L}c", bufs=1) as lpool, \
                     tc.tile_pool(name=f"g{L}b", bufs=3) as spool, \
                     tc.tile_pool(name=f"g{L}g", bufs=3) as gpool, \
                     tc.tile_pool(name=f"g{L}v", bufs=2, space="PSUM") as vpool, \
                     tc.tile_pool(name=f"g{L}ps", bufs=2, space="PSUM") as ppool:
                    half_rows = rows_total // 2
                    # att row materialized across subgroups once per layer so
                    # the per-span z multiply runs as a plain contiguous TT
                    attB = lpool.tile([128, NSG, HC], F16)
                    nc.vector.tensor_copy(
                        out=attB[:],
                        in_=att_rep[:].rearrange(
                            "p (o c) -> p o c", o=1
                        ).broadcast_to((128, NSG, HC)),
                    )
                    for s in range(nspans):
                        midx = spool.tile([128, MI_X1], I16, tag="midx")
                        nc.sync.dma_start(out=midx[:], in_=t_midx[s, :, :])
                        mval = spool.tile([128, MV_W], F16, tag="mval")
                        nc.sync.dma_start(out=mval[:], in_=t_mval[s, :, :])
                        eaT_t = spool.tile([1, SPAN_EDGES], F16, tag="eaT")
                        nc.sync.dma_start(out=eaT_t[:], in_=t_eaT[s:s + 1, :])
                        dcol = spool.tile([128, NSG], F32, tag="dcolF")
                        nc.vector.tensor_copy(out=dcol[:],
                                              in_=mval[:, MV_D0:MV_D1])
                        # G = xl[src] (two half-table gathers), R = xr[dst]
                        # (per-core fp8 xr table gather via packed xr_idx;
                        # fp8 halves the random-read HBM bytes and R only
                        # feeds the attention logits)
                        G = gpool.tile([128, NSG, HC], F8, tag="G")
                        nc.gpsimd.dma_gather(
                            G[:, 0:NSG_H, :], xl_tbl[0:half_rows, :],
                            midx[:, MI_A0:MI_A1],
                            SLOT_H, SLOT_H, HC, single_packet=single_packet,
                            queue_num=0,
                        )
                        nc.gpsimd.dma_gather(
                            G[:, NSG_H:NSG, :], xl_tbl[half_rows:, :],
                            midx[:, MI_B0:MI_B1],
                            SLOT_H, SLOT_H, HC, single_packet=single_packet,
                            queue_num=1 % nqueues,
                        )
                        R = gpool.tile([128, NSG, HC], F8, tag="R")
                        nc.gpsimd.dma_gather(
                            R[:, 0:NSG_H, :], xr_tbl[:, :],
                            midx[:, MI_X0:MI_X0 + SLOT_H // 16],
                            SLOT_H, SLOT_H, HC,
                            single_packet=single_packet,
                            queue_num=2 % nqueues,
                        )
                        nc.gpsimd.dma_gather(
                            R[:, NSG_H:NSG, :], xr_tbl[:, :],
                            midx[:, MI_X0 + SLOT_H // 16:MI_X1],
                            SLOT_H, SLOT_H, HC,
                            single_packet=single_packet,
                            queue_num=3 % nqueues,
                        )
                        # dst one-hot S (for the segment-sum matmuls)
                        S = pool.tile([128, NSG, 128], F16, tag="S")
                        for sg in range(NSG):
                            nc.vector.tensor_scalar(
                                out=S[:, sg, :], in0=iota_rep[:],
                                scalar1=dcol[:, sg:sg + 1], scalar2=None,
                                op0=ALU.is_equal,
                            )
                        if op_limit < 2:
                            hOut = spool.tile([128, HC], F16, tag="hOut")
                            nc.vector.tensor_copy(out=hOut[:], in_=G[:, 0, :])
                            h_sink(s, hOut, mval, pool, ppool)
                            continue
                        # v = ea (x) we + G + R accumulated on PE into PSUM
                        # (rank-1 matmul + two identity matmuls per subgroup);
                        # Act reads each PSUM pair directly: u = prelu(v)
                        # (Prelu: same fn as lrelu, but shares the act table
                        # set with Exp -> no table reloads)
                        u = pool.tile([128, NSG, HC], F16, tag="u")
                        q0 = 0
                        while q0 < NSG:
                            qn = min(2, NSG - q0)
                            accV = vpool.tile([128, 2, HC], F32, tag="accV")
                            for j in range(qn):
                                sg = q0 + j
                                nc.tensor.matmul(
                                    out=accV[:, j, :],
                                    lhsT=eaT_t[0:1, sg * 128:(sg + 1) * 128],
                                    rhs=we_rep[0:1, :], start=True, stop=False)
                                nc.tensor.matmul(
                                    out=accV[:, j, :], lhsT=ident8[:],
                                    rhs=G[:, sg, :], start=False, stop=False)
                                nc.tensor.matmul(
                                    out=accV[:, j, :], lhsT=ident8[:],
                                    rhs=R[:, sg, :], start=False, stop=True)
                            nc.scalar.activation(
                                out=u[:, q0:q0 + qn, :], in_=accV[:, 0:qn, :],
                                func=AF.Prelu, alpha=0.2)
                            q0 += qn

                        if t_udump is not None and L == 1 and s == 0:
                            nc.sync.dma_start(
                                out=t_udump[:, :],
                                in_=u[:].rearrange("p s c -> p (s c)"))
                            nc.sync.dma_start(
                                out=t_gdump[:, :],
                                in_=G[:].rearrange("p s c -> p (s c)"))
                        if op_limit < 3:
                            hOut = spool.tile([128, HC], F16, tag="hOut")
                            nc.vector.tensor_copy(out=hOut[:], in_=u[:, 0, :])
                            h_sink(s, hOut, mval, pool, ppool)
                            continue
                        # z = u*att, alpha = per-head sum
                        z = pool.tile([128, NSG, HC], F16, tag="z")
                        nc.vector.tensor_tensor(
                            out=z[:, :, :], in0=u[:, :, :], in1=attB[:],
                            op=ALU.mult
                        )
                        # per-head sums via binary fold tree
                        zf = pool.tile([128, NSG, 4, 32], F16, tag="zf")
                        z4 = z[:].rearrange("p s (h c) -> p s h c", h=4)
                        nc.vector.tensor_tensor(
                            out=zf[:, :, :, :], in0=z4[:, :, :, 0:32],
                            in1=z4[:, :, :, 32:64], op=ALU.add,
                        )
                        w = 16
                        while w >= 2:
                            nc.vector.tensor_tensor(
                                out=zf[:, :, :, 0:w], in0=zf[:, :, :, 0:w],
                                in1=zf[:, :, :, w:2 * w], op=ALU.add,
                            )
                            w //= 2
                        alpha = spool.tile([128, 4 * NSG], F32, tag="alpha")
                        nc.vector.tensor_tensor(
                            out=alpha[:].rearrange("p (s h o) -> p s h o",
                                                   h=4, o=1),
                            in0=zf[:, :, :, 0:1], in1=zf[:, :, :, 1:2],
                            op=ALU.add,
                        )
                        # exB = exp(alpha) broadcast-materialized over the C
                        # dim in one Act instruction: m2 then hits DVE 2x mode
                        exB = pool.tile([128, NSG, 4, C], F16, tag="exB")
                        nc.scalar.activation(
                            out=exB[:, :, :, :],
                            in_=alpha[:].rearrange(
                                "p (s h o) -> p s h o", h=4, o=1
                            ).broadcast_to((128, NSG, 4, C)),
                            func=AF.Exp,
                        )

                        if op_limit < 4:
                            hOut = spool.tile([128, HC], F16, tag="hOut")
                            nc.vector.tensor_copy(out=hOut[:], in_=u[:, 0, :])
                            nc.vector.tensor_scalar(
                                out=hOut[:, 0:4], in0=exB[:, 0, 0:4, 0],
                                scalar1=1.0,
                                scalar2=None, op0=ALU.mult)
                            h_sink(s, hOut, mval, pool, ppool)
                            continue
                        if t_adump is not None and L == 1 and s == 0:
                            nc.sync.dma_start(out=t_adump[:, :], in_=alpha[:])
                            nc.sync.dma_start(
                                out=t_edump[:, :],
                                in_=exB[:].rearrange("p s h c -> p (s h c)"))
                        # m2 = ex * G (softmax-weighted source messages;
                        # out = sum a*xl[src] directly, no xr/we correction)
                        m2 = pool.tile([128, NSG, HC], F16, tag="m2")
                        nc.vector.tensor_tensor(
                            out=m2[:],
                            in0=G[:].rearrange("p s (h c) -> p s h c", h=4),
                            in1=exB[:, :, :, :],
                            op=ALU.mult,
                        )
                        if t_mdump is not None and L == 1 and s == 0:
                            nc.sync.dma_start(
                                out=t_mdump[:, :],
                                in_=m2[:].rearrange("p s c -> p (s c)"))
                        accM = ppool.tile([128, HC], F32, tag="accM")
                        accE = ppool.tile([128, 4], F32, tag="accE")
                        for sg in range(NSG):
                            nc.tensor.matmul(out=accM[:],
                                             lhsT=S[:, sg, :],
                                             rhs=m2[:, sg, :], start=(sg == 0),
                                             stop=(sg == NSG - 1))
                            nc.tensor.matmul(out=accE[:],
                                             lhsT=S[:, sg, :],
                                             rhs=exB[:, sg, :, 0],
                                             start=(sg == 0), stop=(sg == NSG - 1))

                        if op_limit < 5:
                            hOut = spool.tile([128, HC], F16, tag="hOut")
                            nc.vector.tensor_copy(out=hOut[:], in_=accM[:])
                            h_sink(s, hOut, mval, pool, ppool)
                            continue
                        if t_acdump is not None and L == 1 and s == 0:
                            acs = spool.tile([128, HC + 4], F32, tag="acdbg")
                            nc.vector.tensor_copy(out=acs[:, 0:HC], in_=accM[:])
                            nc.vector.tensor_copy(out=acs[:, HC:HC + 4],
                                                  in_=accE[:])
                            nc.sync.dma_start(out=t_acdump[:, :], in_=acs[:])
                        # flush: h = relu(accM/den + bias)
                        den = spool.tile([128, 4], F32, tag="den")
                        nc.vector.tensor_scalar(
                            out=den[:], in0=accE[:], scalar1=1e-30,
                            scalar2=None, op0=ALU.add,
                        )
                        rden = spool.tile([128, 4], F32, tag="rden")
                        nc.vector.reciprocal(out=rden[:], in_=den[:])
                        hT = spool.tile([128, HC], F16, tag="hT")
                        for hh in range(4):
                            blks = slice(hh * C, (hh + 1) * C)
                            nc.vector.scalar_tensor_tensor(
                                out=hT[:, blks], in0=accM[:, blks],
                                scalar=rden[:, hh:hh + 1], in1=bias_rep[:, blks],
                                op0=ALU.mult, op1=ALU.add,
                            )
                        hOut = spool.tile([128, HC], F16, tag="hOut")
                        nc.scalar.activation(out=hOut[:], in_=hT[:], func=AF.Relu)
                        h_sink(s, hOut, mval, pool, ppool)

            # layer 1: sink writes h1 to DRAM
            def h1_sink(s, hOut, mval, pool, ppool):
                nc.sync.dma_start(out=t_h1[s * 128:(s + 1) * 128, :], in_=hOut[:])
                if t_h1o is not None:
                    nc.sync.dma_start(out=t_h1o[s * 128:(s + 1) * 128, :],
                                      in_=hOut[:])

            if phase_limit >= 2:
                gat_layer(1, t_xl1, t_xr1, h1_sink)

            # --------------------------------------------------------------
            # Phase 4: xl2 from h1 -> AllGather kickoff -> xr2
            # (xr2 compute overlaps the collective; layer 2's R-gathers are
            # the only consumers of xr2 and start after the table arrives)
            # --------------------------------------------------------------
            with tc.tile_pool(name="p4", bufs=3) as pool, \
                 tc.tile_pool(name="p4ps", bufs=2, space="PSUM") as ppool:
                def xw2(s, nm0, sink0, nm1, sink1):
                    h1T0 = pool.tile([128, 128], F16, tag="h1T0")
                    h1T1 = pool.tile([128, 128], F16, tag="h1T1")
                    nc.sync.dma_start(
                        out=h1T0[:], in_=t_h1[s * 128:(s + 1) * 128, 0:128],
                        transpose=True,
                    )
                    nc.sync.dma_start(
                        out=h1T1[:], in_=t_h1[s * 128:(s + 1) * 128, 128:256],
                        transpose=True,
                    )
                    for nm, sink, dt in ((nm0, sink0, F8), (nm1, sink1, F8)):
                        wa, wb, wc = w2_tiles[nm]
                        ps = ppool.tile([128, HC], F32, tag="ps")
                        nc.tensor.matmul(out=ps[:], lhsT=h1T0[:], rhs=wa[:],
                                         start=True, stop=False)
                        nc.tensor.matmul(out=ps[:], lhsT=h1T1[:], rhs=wb[:],
                                         start=False, stop=False)
                        nc.tensor.matmul(out=ps[:], lhsT=ones_col[:],
                                         rhs=wc[:], start=False, stop=True)
                        xs = pool.tile([128, HC], dt, tag=f"xs{mybir.dt.size(dt)}")
                        nc.vector.tensor_copy(out=xs[:], in_=ps[:])
                        nc.sync.dma_start(out=sink[s * 128:(s + 1) * 128, :],
                                          in_=xs[:])

                for s in range(nspans if phase_limit >= 3 else 0):
                    xw2(s, "wl_aug", t_xl2_in, "wr_aug", t_xr2)

                # ----------------------------------------------------------
                # Phase 5: AllGather xl2
                # ----------------------------------------------------------
                if phase_limit >= 4:
                    nc.gpsimd.collective_compute(
                        "AllGather",
                        ALU.bypass,
                        replica_groups=[list(range(NCORES))],
                        ins=[t_xl2_in.ap().opt()],
                        outs=[t_xl2.ap().opt()],
                    )

            # --------------------------------------------------------------
            # Phase 6: GAT layer 2 with fused pooling
            # --------------------------------------------------------------
            if phase_limit < 5:
                with tc.tile_pool(name="dummyout", bufs=1) as dpool:
                    dz = dpool.tile([4, 64], F32)
                    nc.vector.memset(dz[:], 0.0)
                    nc.sync.dma_start(out=t_out[:], in_=dz[:])
                return
            with tc.tile_pool(name="gpool_ps", bufs=1, space="PSUM") as gpool_ps:
                gpsum = gpool_ps.tile([4, HC], F32)

                def h2_sink(s, hOut, mval, pool, ppool):
                    nc.tensor.matmul(out=gpsum[:], lhsT=mval[:, MV_G0:MV_G1],
                                     rhs=hOut[:],
                                     start=(s == 0), stop=(s == nspans - 1))

                gat_layer(2, t_xl2, t_xr2, h2_sink)

                # ----------------------------------------------------------
                # Phase 7: pooling -> MLP -> out
                # ----------------------------------------------------------
                with tc.tile_pool(name="mlp", bufs=1) as pool, \
                     tc.tile_pool(name="mlp_ps", bufs=2, space="PSUM") as ppool:
                    g = pool.tile([4, HC], F32)
                    nc.vector.tensor_scalar(out=g[:], in0=gpsum[:],
                                            scalar1=icnt[:, 0:1],
                                            scalar2=None, op0=ALU.mult)
                    # gT via PE transpose (two halves)
                    gT = pool.tile([128, 8], F32)
                    for half in range(2):
                        tp = ppool.tile([128, 128], F32, tag="tp")
                        nc.tensor.transpose(
                            out=tp[:, 0:4],
                            in_=g[:, half * 128:(half + 1) * 128],
                            identity=ident[0:4, 0:4],
                        )
                        nc.vector.tensor_copy(out=gT[:, half * 4:half * 4 + 4],
                                              in_=tp[:, 0:4])
                    z1p = ppool.tile([4, 128], F32, tag="z1p")
                    nc.tensor.matmul(out=z1p[:], lhsT=gT[:, 0:4], rhs=p1a[:],
                                     start=True, stop=False)
                    nc.tensor.matmul(out=z1p[:], lhsT=gT[:, 4:8], rhs=p1b[:],
                                     start=False, stop=False)
                    nc.tensor.matmul(out=z1p[:], lhsT=onesg[:], rhs=p1c[:],
                                     start=False, stop=True)
                    z1 = pool.tile([4, 128], F32)
                    nc.vector.tensor_copy(out=z1[:], in_=z1p[:])
                    # layernorm over free dim (128)
                    mu = pool.tile([4, 1], F32)
                    nc.vector.reduce_sum(out=mu[:], in_=z1[:], axis=AXX)
                    nc.vector.tensor_scalar(out=mu[:], in0=mu[:],
                                            scalar1=1.0 / 128,
                                            scalar2=None, op0=ALU.mult)
                    zc = pool.tile([4, 128], F32)
                    nc.vector.tensor_scalar(out=zc[:], in0=z1[:],
                                            scalar1=mu[:, 0:1],
                                            scalar2=None, op0=ALU.subtract)
                    sq = pool.tile([4, 128], F32)
                    nc.vector.tensor_tensor(out=sq[:], in0=zc[:], in1=zc[:],
                                            op=ALU.mult)
                    var = pool.tile([4, 1], F32)
                    nc.vector.reduce_sum(out=var[:], in_=sq[:], axis=AXX)
                    nc.vector.tensor_scalar(out=var[:], in0=var[:],
                                            scalar1=1.0 / 128,
                                            scalar2=1e-5, op0=ALU.mult,
                                            op1=ALU.add)
                    std = pool.tile([4, 1], F32)
                    nc.scalar.activation(out=std[:], in_=var[:], func=AF.Sqrt)
                    rstd = pool.tile([4, 1], F32)
                    nc.vector.reciprocal(out=rstd[:], in_=std[:])
                    zn = pool.tile([4, 128], F32)
                    nc.vector.tensor_scalar(out=zn[:], in0=zc[:],
                                            scalar1=rstd[:, 0:1],
                                            scalar2=None, op0=ALU.mult)
                    nc.vector.tensor_tensor(out=zn[:], in0=zn[:], in1=lng[:],
                                            op=ALU.mult)
                    nc.vector.tensor_tensor(out=zn[:], in0=zn[:], in1=lnb[:],
                                            op=ALU.add)
                    nc.scalar.activation(out=zn[:], in_=zn[:], func=AF.Relu)
                    # z2 = relu(zn @ p2 + b2)
                    znT = pool.tile([128, 4], F32)
                    tp2 = ppool.tile([128, 128], F32, tag="tp")
                    nc.tensor.transpose(out=tp2[:, 0:4], in_=zn[:],
                                        identity=ident[0:4, 0:4])
                    nc.vector.tensor_copy(out=znT[:], in_=tp2[:, 0:4])
                    z2p = ppool.tile([4, 64], F32, tag="z2p")
                    nc.tensor.matmul(out=z2p[:], lhsT=znT[:], rhs=p2a[:],
                                     start=True, stop=False)
                    nc.tensor.matmul(out=z2p[:], lhsT=onesg[:], rhs=p2c[:],
                                     start=False, stop=True)
                    zout = pool.tile([4, 64], F32)
                    nc.scalar.activation(out=zout[:], in_=z2p[:], func=AF.Relu)
                    nc.sync.dma_start(out=t_out[:], in_=zout[:])

        for _rep in range(reps):
            build_rep()

    nc.finalize()
    # Tile assigns SWDGE completion-sem lanes (8) round-robin over Pool DMA
    # insts in SCHEDULED order, and each lane must stick to one queue.  The
    # scheduler reorders gathers across spans, so rewrite queue_num in
    # scheduled order: lane i%8 <-> queue i%nqueues (nqueues | 8).
    i = 0
    for blk in nc.m.functions[0].blocks:
        for inst in blk.instructions:
            if inst.engine == mybir.EngineType.Pool and isinstance(
                inst, mybir.InstDMAGatherAnt
            ):
                inst.queue_num = i % nqueues
                i += 1
    return nc


# ----------------------------------------------------------------------------
# Entry point
# ----------------------------------------------------------------------------

def _pack_inputs(inp, cores, packs, nspans, rows_per_core, rows_total, x_aug_T):
    f16 = np.float16
    f32 = np.float32
    # shared (replicated) blocks
    wpack = np.zeros((WP_ROWS, HC), dtype=f16)

    def aug(w, b):
        return np.concatenate(
            [np.asarray(w, f32), np.asarray(b, f32)[None, :]], 0
        ).astype(f16)

    wpack[WP_WL1:WP_WL1 + 65] = aug(inp["g1_wl"], inp["g1_bl"])
    wpack[WP_WR1:WP_WR1 + 65] = aug(inp["g1_wr"], inp["g1_br"])
    wpack[WP_WL2:WP_WL2 + 257] = aug(inp["g2_wl"], inp["g2_bl"])
    wpack[WP_WR2:WP_WR2 + 257] = aug(inp["g2_wr"], inp["g2_br"])
    for L, (r_att, r_we, r_bias) in (
        (1, (WP_ATT1, WP_WE1, WP_BIAS1)),
        (2, (WP_ATT2, WP_WE2, WP_BIAS2)),
    ):
        wpack[r_att:r_att + 128] = np.broadcast_to(
            np.asarray(inp[f"g{L}_att"], f32).reshape(1, HC), (128, HC)
        ).astype(f16)
        wpack[r_we:r_we + 128] = np.broadcast_to(
            np.asarray(inp[f"g{L}_we"], f32).reshape(1, HC), (128, HC)
        ).astype(f16)
        wpack[r_bias:r_bias + 128] = np.broadcast_to(
            np.asarray(inp[f"g{L}_bias"], f32).reshape(1, HC), (128, HC)
        ).astype(f16)
    wpack[WP_IOTA:WP_IOTA + 128, 0:128] = np.broadcast_to(
        np.arange(128, dtype=f16)[None, :], (128, 128)
    )

    mlp = np.zeros((MP_ROWS, 128), dtype=f32)
    mlp[MP_P1:MP_P1 + 257] = np.concatenate(
        [np.asarray(inp["p1_w"], f32), np.asarray(inp["p1_b"], f32)[None, :]], 0
    )
    mlp[MP_LNG:MP_LNG + 4] = np.asarray(inp["ln_g"], f32)[None, :]
    mlp[MP_LNB:MP_LNB + 4] = np.asarray(inp["ln_b"], f32)[None, :]
    mlp[MP_P2:MP_P2 + 129, 0:64] = np.concatenate(
        [np.asarray(inp["p2_w"], f32), np.asarray(inp["p2_b"], f32)[None, :]], 0
    )

    enc_aug = np.concatenate(
        [np.asarray(inp["enc_w"], f32), np.asarray(inp["enc_b"], f32)[None, :]], 0
    )  # [5, 64]

    in_maps = []
    for k in range(NCORES):
        p = packs[k]
        feat = np.zeros((5, rows_total + rows_per_core + 64), dtype=f32)
        feat[:, 0:rows_total] = x_aug_T
        feat[:, rows_total:rows_total + rows_per_core] = x_aug_T[:, p["own_cols"]]
        feat[:, rows_total + rows_per_core:] = enc_aug
        mlp_k = mlp.copy()
        mlp_k[MP_CNT:MP_CNT + 4, 0] = p["inv_cnt"]
        in_maps.append({
            "feat": feat,
            "wpack": wpack,
            "meta_idx": p["meta_idx"],
            "meta_val": p["meta_val"].view(np.float16),
            "eaT": p["ea_T"],
            "mlp": mlp_k,
        })
    return in_maps


def kernel(**inputs):
    cores, packs, nspans, rows_per_core, rows_total, x_aug_T, node_row = _host_prep(
        inputs
    )
    key = (nspans, rows_total)
    if key not in _PROGRAM_CACHE:
        _PROGRAM_CACHE[key] = _build_program(nspans, rows_total)
    nc = _PROGRAM_CACHE[key]
    in_maps = _pack_inputs(
        inputs, cores, packs, nspans, rows_per_core, rows_total, x_aug_T
    )
    res = run_bass_kernel_spmd(nc, in_maps, core_ids=list(range(NCORES)))
    out = np.concatenate([res.results[k]["out"] for k in range(NCORES)], axis=0)
    return out.astype(np.float32)


if __name__ == "__main__":
    data = dict(np.load("/root/problem/inputs_cache.npz"))
    out = kernel(**data)
    exp = np.load("/root/problem/expected_np.npy")
    rel = np.linalg.norm(out - exp) / np.linalg.norm(exp)
    print("rel err:", rel)



# revision 32
# speedup vs baseline: 1.2110x; 1.2110x over previous
"""Trainium2 Bass kernel for nn_GATv2Base (gnn_message_passing).

Contract: kernel(**inputs) takes FULL unsharded inputs (same keys as
reference.setup_inputs()) and returns the FULL [32, 64] float32 output.

Sharding: 32 graphs -> 8 cores (4 graphs each, contiguous node ranges since
`batch` is sorted).  Edges (plus self-loops) are routed to the core owning
their dst node, sorted by dst, and packed into "spans" (<=127-node dst window,
2304 edge slots = 18 subgroups of 128 edges).  Node features live in a
span-major padded global layout so every per-span device address is static.
Layer 1 runs fully local (xl1 table computed replicated from x); between
layers one fp16 AllGather shares the xl2 table; the pooled per-graph MLP is
computed on the owning core.  Only the [4, 64] per-core outputs return to the
host.

All per-core inputs are packed into 5 device tensors (feat/wpack/meta_idx/
meta_val/mlp) to minimize per-call argument-marshalling overhead on the host
runtime.  _build_program(reps=N) unrolls the whole computation N times inside
one program; timing two variants isolates true device execution time from
dispatch overhead.
"""

import os
import sys

import numpy as np

for _p in ("/opt/trn_rl_repo", "/root/.axon_site/_ro/trn_rl_repo"):
    if os.path.isdir(_p) and _p not in sys.path:
        sys.path.insert(0, _p)

import concourse.bass as bass
import concourse.bacc as bacc
import concourse.mybir as mybir
import concourse.tile as tile
from concourse.bass_utils import run_bass_kernel_spmd

F32 = mybir.dt.float32
F16 = mybir.dt.float16
F8 = mybir.dt.float8e4
I16 = mybir.dt.int16
I32 = mybir.dt.int32
AF = mybir.ActivationFunctionType
ALU = mybir.AluOpType
AXX = mybir.AxisListType.X

N, E, H, C, NG = 50000, 800000, 4, 64, 32
HC = H * C
NCORES = 8
SLOT_H = 1152            # edge slots per table-half region (9 subgroups)
SPAN_EDGES = 2 * SLOT_H  # 2304 edge slots per span (18 subgroups of 128)
NSG = SPAN_EDGES // 128  # 18
NSG_H = SLOT_H // 128    # 9
SPAN_DST = 127           # dst window per span; slot 127 = pad marker

# meta_idx layout (i16 cols): src_idx_a 0:72, src_idx_b 72:144, xr_idx 144:288
MI_A0, MI_A1 = 0, SLOT_H // 16
MI_B0, MI_B1 = MI_A1, 2 * (SLOT_H // 16)
MI_X0, MI_X1 = MI_B1, MI_B1 + SPAN_EDGES // 16
# meta_val layout (f16 cols): dcol 0:18, eac 18:36, gmask 36:40
MV_D0, MV_D1 = 0, NSG
MV_E0, MV_E1 = NSG, 2 * NSG
MV_G0, MV_G1 = 2 * NSG, 2 * NSG + 4
MV_W = MV_G1
# wpack rows (f16, width 256)
WP_WL1, WP_WR1 = 0, 65
WP_ATT1, WP_WE1, WP_BIAS1 = 130, 258, 386
WP_WL2, WP_WR2 = 514, 771
WP_ATT2, WP_WE2, WP_BIAS2 = 1028, 1156, 1284
WP_IOTA = 1412
WP_ROWS = 1540
# mlp rows (f32, width 128)
MP_P1, MP_LNG, MP_LNB, MP_P2, MP_CNT = 0, 257, 261, 265, 394
MP_ROWS = 398


# ----------------------------------------------------------------------------
# Host-side sharding / packing
# ----------------------------------------------------------------------------

def _host_prep(inp):
    x = np.asarray(inp["x"], dtype=np.float32)
    ei = np.asarray(inp["edge_index"], dtype=np.int32)
    ea_full = np.asarray(inp["edge_attr"], dtype=np.float32)[:, 0]
    batch = np.asarray(inp["batch"], dtype=np.int32)

    src0, dst0 = ei[0], ei[1]
    deg = np.maximum(np.bincount(dst0, minlength=N).astype(np.float64), 1.0)
    loop_attr = (
        np.bincount(dst0, weights=ea_full.astype(np.float64), minlength=N) / deg
    ).astype(np.float32)
    src = np.concatenate([src0, np.arange(N, dtype=np.int32)])
    dst = np.concatenate([dst0, np.arange(N, dtype=np.int32)])
    eattr = np.concatenate([ea_full, loop_attr]).astype(np.float32)

    gcounts = np.bincount(batch, minlength=NG)
    gstart = np.concatenate([[0], np.cumsum(gcounts)])
    core_n0 = np.array([gstart[4 * k] for k in range(NCORES)] + [N], dtype=np.int64)

    order = np.argsort(dst, kind="stable")
    src, dst, eattr = src[order], dst[order], eattr[order]
    edge_lo = np.searchsorted(dst, core_n0[:-1], "left")
    edge_hi = np.searchsorted(dst, core_n0[1:], "left")

    # src owner core (cores 0-3 -> table half A, 4-7 -> half B); stable
    # under span-count changes so it can drive packing.
    src_owner = np.searchsorted(core_n0[1:], src, "right")
    src_in_a = src_owner < (NCORES // 2)

    cores = []
    for k in range(NCORES):
        n0, n1 = int(core_n0[k]), int(core_n0[k + 1])
        s, e = int(edge_lo[k]), int(edge_hi[k])
        cd = dst[s:e]
        ca = src_in_a[s:e]
        nlocal = n1 - n0
        node_edge_start = np.searchsorted(cd, n0 + np.arange(nlocal + 1))
        cumA = np.concatenate([[0], np.cumsum(ca)])  # over edges
        spans = []
        b = 0
        while b < nlocal:
            bend = b
            while bend < nlocal and (bend - b) < SPAN_DST:
                e0, e1 = node_edge_start[b], node_edge_start[bend + 1]
                nA = cumA[e1] - cumA[e0]
                nB = (e1 - e0) - nA
                if nA > SLOT_H or nB > SLOT_H:
                    break
                bend += 1
            assert bend > b, "single node exceeds span edge capacity"
            spans.append(
                (b, bend - b, int(node_edge_start[b]), int(node_edge_start[bend]))
            )
            b = bend
        cores.append(
            dict(n0=n0, n1=n1, spans=spans, src=src[s:e], dst=cd, ea=eattr[s:e],
                 in_a=ca)
        )

    nspans = max(len(c["spans"]) for c in cores)
    rows_per_core = nspans * 128
    rows_total = NCORES * rows_per_core

    # global padded row per node
    node_row = np.zeros(N, dtype=np.int64)
    for k, c in enumerate(cores):
        for si, (b, nb, _, _) in enumerate(c["spans"]):
            nodes = np.arange(c["n0"] + b, c["n0"] + b + nb)
            node_row[nodes] = k * rows_per_core + si * 128 + (nodes - c["n0"] - b)

    # x padded, transposed, with ones row (for encoder rhs)
    x_pad = np.zeros((rows_total, 4), dtype=np.float32)
    x_pad[node_row] = x
    x_aug_T = np.concatenate(
        [x_pad.T, np.ones((1, rows_total), dtype=np.float32)], axis=0
    )  # [5, R]

    packs = []
    half_rows = rows_total // 2
    assert half_rows <= 32767, f"table half {half_rows} exceeds int16 index range"

    def wrap_idx16(vals):
        # vals: [SLOT] int -> wrapped [128, SLOT//16] int16 (16-part wrap,
        # replicated over the 8 q7 core groups)
        slot = len(vals)
        base = np.zeros((16, slot // 16), dtype=np.int16)
        i = np.arange(slot)
        base[i % 16, i // 16] = vals.astype(np.int16)
        return np.tile(base, (8, 1))

    for k, c in enumerate(cores):
        meta_idx = np.zeros((nspans, 128, MI_X1), dtype=np.int16)
        meta_val = np.zeros((nspans, 128, MV_W), dtype=np.float16)
        ea_T = np.zeros((nspans, SPAN_EDGES), dtype=np.float16)
        meta_val[:, :, MV_D0:MV_D1] = np.float16(127.0)
        for si, (b, nb, e0, e1) in enumerate(c["spans"]):
            ina = c["in_a"][e0:e1]
            esrc = node_row[c["src"][e0:e1]]
            edrel = (c["dst"][e0:e1] - c["n0"] - b).astype(np.int64)
            eea = c["ea"][e0:e1]
            # slots: A edges first (in region [0, SLOT_H)), then B edges at
            # [SLOT_H, 2*SLOT_H); pads keep idx 0 / drel 127 / ea 0
            ia = np.where(ina)[0]
            ib = np.where(~ina)[0]
            slots = np.empty(len(ina), dtype=np.int64)
            slots[ia] = np.arange(len(ia))
            slots[ib] = SLOT_H + np.arange(len(ib))
            av = np.zeros(SLOT_H, dtype=np.int64)
            av[:len(ia)] = esrc[ia]
            bv = np.zeros(SLOT_H, dtype=np.int64)
            bv[:len(ib)] = esrc[ib] - half_rows
            meta_idx[si, :, MI_A0:MI_A1] = wrap_idx16(av)
            meta_idx[si, :, MI_B0:MI_B1] = wrap_idx16(bv)
            xv = np.full(SPAN_EDGES, si * 128 + 127, dtype=np.int64)
            xv[slots] = si * 128 + edrel
            meta_idx[si, :, MI_X0:MI_X1] = wrap_idx16(xv)
            p, sg = slots % 128, slots // 128
            meta_val[si, p, MV_D0 + sg] = edrel.astype(np.float16)
            meta_val[si, p, MV_E0 + sg] = eea.astype(np.float16)
            ea_T[si, slots] = eea.astype(np.float16)
            nodes = np.arange(c["n0"] + b, c["n0"] + b + nb)
            gl = batch[nodes] - 4 * k
            meta_val[si, np.arange(nb), MV_G0 + gl] = np.float16(1.0)
        inv_cnt = np.zeros((4,), dtype=np.float32)
        for gg in range(4):
            cnt = max(int(gcounts[4 * k + gg]), 1)
            inv_cnt[gg] = 1.0 / cnt
        packs.append(
            dict(
                meta_idx=meta_idx,
                meta_val=meta_val,
                ea_T=ea_T,
                inv_cnt=inv_cnt,
                own_cols=np.arange(
                    k * rows_per_core, (k + 1) * rows_per_core, dtype=np.int64
                ),
            )
        )
    return cores, packs, nspans, rows_per_core, rows_total, x_aug_T, node_row


# ----------------------------------------------------------------------------
# Device program
# ----------------------------------------------------------------------------

_PROGRAM_CACHE = {}


def _build_program(nspans, rows_total, reps=1, phase_limit=5, op_limit=9,
                   single_packet=False, nqueues=4, skip_r=False,
                   debug_h1=False):
    rows_per_core = nspans * 128
    nblocks = rows_total // 128

    nc = bacc.Bacc(num_swdge_queues=nqueues)
    tcx = tile.TileContext(nc)

    t_feat = nc.dram_tensor(
        "feat", [5, rows_total + rows_per_core + 64], F32, kind="ExternalInput"
    )
    t_wpack = nc.dram_tensor("wpack", [WP_ROWS, HC], F16, kind="ExternalInput")
    t_midx = nc.dram_tensor(
        "meta_idx", [nspans, 128, MI_X1], I16, kind="ExternalInput"
    )
    t_mval = nc.dram_tensor(
        "meta_val", [nspans, 128, MV_W], F16, kind="ExternalInput"
    )
    t_eaT = nc.dram_tensor("eaT", [nspans, SPAN_EDGES], F16, kind="ExternalInput")
    t_mlp = nc.dram_tensor("mlp", [MP_ROWS, 128], F32, kind="ExternalInput")
    t_out = nc.dram_tensor("out", [4, 64], F32, kind="ExternalOutput")

    # ---- internal DRAM ----
    t_xl1 = nc.dram_tensor("xl1_tbl", [rows_total, HC], F16)
    t_xr1 = nc.dram_tensor("xr1_own", [rows_per_core, HC], F8)
    t_h1 = nc.dram_tensor("h1_own", [rows_per_core, HC], F16)
    t_h1o = (nc.dram_tensor("h1dump", [rows_per_core, HC], F16,
                            kind="ExternalOutput") if debug_h1 else None)
    t_udump = (nc.dram_tensor("udump", [128, NSG * HC], F16,
                              kind="ExternalOutput") if debug_h1 else None)
    t_gdump = (nc.dram_tensor("gdump", [128, NSG * HC], F16,
                              kind="ExternalOutput") if debug_h1 else None)
    t_adump = (nc.dram_tensor("adump", [128, 4 * NSG], F32,
                              kind="ExternalOutput") if debug_h1 else None)
    t_edump = (nc.dram_tensor("edump", [128, NSG * HC], F16,
                              kind="ExternalOutput") if debug_h1 else None)
    t_acdump = (nc.dram_tensor("acdump", [128, HC + 4], F32,
                               kind="ExternalOutput") if debug_h1 else None)
    t_mdump = (nc.dram_tensor("mdump", [128, NSG * HC], F16,
                              kind="ExternalOutput") if debug_h1 else None)
    t_xr2 = nc.dram_tensor("xr2_own", [rows_per_core, HC], F8)
    t_xl2_in = nc.dram_tensor("xl2_own_cc", [rows_per_core, HC], F16)
    t_xl2 = nc.dram_tensor("xl2_tbl", [rows_total, HC], F16, addr_space="Shared")

    from contextlib import ExitStack
    with tcx as tc, ExitStack() as es:
        # ------------------------------------------------------------------
        # constants in SBUF (loaded once, reused by every rep)
        # ------------------------------------------------------------------
        cpool = es.enter_context(tc.tile_pool(name="consts", bufs=1))
        enc_aug = cpool.tile([5, 64], F32)
        nc.sync.dma_start(
            out=enc_aug[:],
            in_=t_feat[:, rows_total + rows_per_core:rows_total + rows_per_core + 64],
        )
        iota_rep = cpool.tile([128, 128], F16)
        nc.sync.dma_start(out=iota_rep[:], in_=t_wpack[WP_IOTA:WP_IOTA + 128, 0:128])
        reps_t = {}
        for L, (r_att, r_we, r_bias) in (
            (1, (WP_ATT1, WP_WE1, WP_BIAS1)),
            (2, (WP_ATT2, WP_WE2, WP_BIAS2)),
        ):
            for nm, r0 in (("att_row", r_att), ("we_row", r_we), ("bias_row", r_bias)):
                rep = cpool.tile([128, HC], F16, tag=f"rep{L}{nm}")
                nc.sync.dma_start(out=rep[:], in_=t_wpack[r0:r0 + 128, :])
                reps_t[(L, nm)] = rep
        ones_col = cpool.tile([1, 128], F16)
        nc.vector.memset(ones_col[:], 1.0)
        ones_row = cpool.tile([1, 512], F16)
        nc.vector.memset(ones_row[:], 1.0)
        from concourse.masks import make_identity
        ident16 = cpool.tile([128, 128], F16)
        make_identity(nc, ident16[:])
        ident8 = cpool.tile([128, 128], F8)
        nc.vector.tensor_copy(out=ident8[:], in_=ident16[:])

        wpool = es.enter_context(tc.tile_pool(name="weights", bufs=1))
        wl1 = wpool.tile([65, HC], F16)
        wr1 = wpool.tile([65, HC], F16)
        nc.sync.dma_start(out=wl1[:], in_=t_wpack[WP_WL1:WP_WL1 + 65, :])
        nc.sync.dma_start(out=wr1[:], in_=t_wpack[WP_WR1:WP_WR1 + 65, :])
        w2_tiles = {}
        for nm, r0 in (("wl_aug", WP_WL2), ("wr_aug", WP_WR2)):
            a = wpool.tile([128, HC], F16, tag=f"{nm}a")
            b = wpool.tile([128, HC], F16, tag=f"{nm}b")
            cbias = wpool.tile([1, HC], F16, tag=f"{nm}c")
            nc.sync.dma_start(out=a[:], in_=t_wpack[r0:r0 + 128, :])
            nc.sync.dma_start(out=b[:], in_=t_wpack[r0 + 128:r0 + 256, :])
            nc.sync.dma_start(out=cbias[:], in_=t_wpack[r0 + 256:r0 + 257, :])
            w2_tiles[nm] = (a, b, cbias)
        # MLP constants
        mpool = es.enter_context(tc.tile_pool(name="mlpc", bufs=1))
        p1a = mpool.tile([128, 128], F32)
        p1b = mpool.tile([128, 128], F32)
        p1c = mpool.tile([1, 128], F32)
        nc.sync.dma_start(out=p1a[:], in_=t_mlp[MP_P1:MP_P1 + 128, :])
        nc.sync.dma_start(out=p1b[:], in_=t_mlp[MP_P1 + 128:MP_P1 + 256, :])
        nc.sync.dma_start(out=p1c[:], in_=t_mlp[MP_P1 + 256:MP_P1 + 257, :])
        p2a = mpool.tile([128, 64], F32)
        p2c = mpool.tile([1, 64], F32)
        nc.sync.dma_start(out=p2a[:], in_=t_mlp[MP_P2:MP_P2 + 128, 0:64])
        nc.sync.dma_start(out=p2c[:], in_=t_mlp[MP_P2 + 128:MP_P2 + 129, 0:64])
        lng = mpool.tile([4, 128], F32)
        nc.sync.dma_start(out=lng[:], in_=t_mlp[MP_LNG:MP_LNG + 4, :])
        lnb = mpool.tile([4, 128], F32)
        nc.sync.dma_start(out=lnb[:], in_=t_mlp[MP_LNB:MP_LNB + 4, :])
        icnt = mpool.tile([4, 1], F32)
        nc.sync.dma_start(out=icnt[:], in_=t_mlp[MP_CNT:MP_CNT + 4, 0:1])
        ident = mpool.tile([128, 128], F32)
        from concourse.masks import make_identity
        make_identity(nc, ident[:])
        onesg = mpool.tile([1, 4], F32)
        nc.vector.memset(onesg[:], 1.0)

        def encode4(pool, ppool, col0, ncols):
            """Encode ncols (<=512) padded nodes starting at feat col col0.
            Returns h0T4 [65, ncols] f16 (aug ones row included)."""
            xT = pool.tile([5, 512], F32, tag="xT")
            nc.sync.dma_start(out=xT[:, 0:ncols], in_=t_feat[:, col0:col0 + ncols])
            h0p = ppool.tile([64, 512], F32, tag="h0ps")
            nc.tensor.matmul(out=h0p[:, 0:ncols], lhsT=enc_aug[:],
                             rhs=xT[:, 0:ncols], start=True, stop=True)
            h0T = pool.tile([65, 512], F16, tag="h0T")
            nc.scalar.activation(out=h0T[0:64, 0:ncols], in_=h0p[:, 0:ncols],
                                 func=AF.Relu)
            nc.vector.tensor_copy(out=h0T[64:65, 0:ncols],
                                  in_=ones_row[:, 0:ncols])
            return h0T

        def xw_blocks(pool, ppool, h0T, w, nblk, sink_ap, dt=F16):
            """nblk xl/xr matmuls from h0T slices; one batched DMA to DRAM."""
            xls = pool.tile([128, 4, HC], dt, tag=f"xls{mybir.dt.size(dt)}")
            for j in range(nblk):
                xlp = ppool.tile([128, HC], F32, tag="xlps")
                nc.tensor.matmul(out=xlp[:], lhsT=h0T[:, j * 128:(j + 1) * 128],
                                 rhs=w[:], start=True, stop=True)
                if j % 2 == 0:
                    nc.vector.tensor_copy(out=xls[:, j, :], in_=xlp[:])
                else:
                    nc.scalar.copy(out=xls[:, j, :], in_=xlp[:])
            nc.sync.dma_start(
                out=sink_ap.rearrange("(b p) c -> p b c", p=128),
                in_=xls[:, 0:nblk, :],
            )

        def build_rep():
            # --------------------------------------------------------------
            # Phase 1: encoder + xl1 for ALL rows (4 blocks per DMA batch)
            # --------------------------------------------------------------
            with tc.tile_pool(name="p1", bufs=3) as pool, \
                 tc.tile_pool(name="p1ps", bufs=2, space="PSUM") as ppool:
                nb4 = (nblocks + 3) // 4 if phase_limit >= 1 else 0
                for b4 in range(nb4):
                    nblk = min(4, nblocks - b4 * 4)
                    h0T = encode4(pool, ppool, b4 * 512, nblk * 128)
                    xw_blocks(pool, ppool, h0T, wl1, nblk,
                              t_xl1[b4 * 512:b4 * 512 + nblk * 128, :])
                ns4 = (nspans + 3) // 4 if phase_limit >= 1 else 0
                for s4 in range(ns4):
                    nblk = min(4, nspans - s4 * 4)
                    h0T = encode4(pool, ppool, rows_total + s4 * 512, nblk * 128)
                    xw_blocks(pool, ppool, h0T, wr1, nblk,
                              t_xr1[s4 * 512:s4 * 512 + nblk * 128, :], dt=F8)

            # --------------------------------------------------------------
            # GAT span loop (shared for both layers)
            # --------------------------------------------------------------
            def gat_layer(L, xl_tbl, xr_tbl, h_sink):
                """h_sink(s, htile, mval): consume flush output [128, HC] f16."""
                att_rep = reps_t[(L, "att_row")]
                we_rep = reps_t[(L, "we_row")]
                bias_rep = reps_t[(L, "bias_row")]
                with tc.tile_pool(name=f"g{L}", bufs=2) as pool, \
                     tc.tile_pool(name=f"g{L}c", bufs=1) as lpool, \
                     tc.tile_pool(name=f"g{L}b", bufs=3) as spool, \
                     tc.tile_pool(name=f"g{L}g", bufs=3) as gpool, \
                     tc.tile_pool(name=f"g{L}v", bufs=2, space="PSUM") as vpool, \
                     tc.tile_pool(name=f"g{L}ps", bufs=2, space="PSUM") as ppool:
                    half_rows = rows_total // 2
                    # att row materialized across subgroups once per layer so
                    # the per-span z multiply runs as a plain contiguous TT
                    attB = lpool.tile([128, NSG, HC], F16)
                    nc.vector.tensor_copy(
                        out=attB[:],
                        in_=att_rep[:].rearrange(
                            "p (o c) -> p o c", o=1
                        ).broadcast_to((128, NSG, HC)),
                    )
                    for s in range(nspans):
                        midx = spool.tile([128, MI_X1], I16, tag="midx")
                        nc.sync.dma_start(out=midx[:], in_=t_midx[s, :, :])
                        mval = spool.tile([128, MV_W], F16, tag="mval")
                        nc.sync.dma_start(out=mval[:], in_=t_mval[s, :, :])
                        eaT_t = spool.tile([1, SPAN_EDGES], F16, tag="eaT")
                        nc.sync.dma_start(out=eaT_t[:], in_=t_eaT[s:s + 1, :])
                        dcol = spool.tile([128, NSG], F32, tag="dcolF")
                        nc.vector.tensor_copy(out=dcol[:],
                                              in_=mval[:, MV_D0:MV_D1])
                        # G = xl[src] (two half-table gathers), R = xr[dst]
                        # (per-core fp8 xr table gather via packed xr_idx;
                        # fp8 halves the random-read HBM bytes and R only
                        # feeds the attention logits)
                        G = gpool.tile([128, NSG, HC], F16, tag="G")
                        nc.gpsimd.dma_gather(
                            G[:, 0:NSG_H, :], xl_tbl[0:half_rows, :],
                            midx[:, MI_A0:MI_A1],
                            SLOT_H, SLOT_H, HC, single_packet=single_packet,
                            queue_num=0,
                        )
                        nc.gpsimd.dma_gather(
                            G[:, NSG_H:NSG, :], xl_tbl[half_rows:, :],
                            midx[:, MI_B0:MI_B1],
                            SLOT_H, SLOT_H, HC, single_packet=single_packet,
                            queue_num=1 % nqueues,
                        )
                        R = gpool.tile([128, NSG, HC], F8, tag="R")
                        nc.gpsimd.dma_gather(
                            R[:, 0:NSG_H, :], xr_tbl[:, :],
                            midx[:, MI_X0:MI_X0 + SLOT_H // 16],
                            SLOT_H, SLOT_H, HC,
                            single_packet=single_packet,
                            queue_num=2 % nqueues,
                        )
                        nc.gpsimd.dma_gather(
                            R[:, NSG_H:NSG, :], xr_tbl[:, :],
                            midx[:, MI_X0 + SLOT_H // 16:MI_X1],
                            SLOT_H, SLOT_H, HC,
                            single_packet=single_packet,
                            queue_num=3 % nqueues,
                        )
                        # dst one-hot S (for the segment-sum matmuls)
                        S = pool.tile([128, NSG, 128], F16, tag="S")
                        for sg in range(NSG):
                            nc.vector.tensor_scalar(
                                out=S[:, sg, :], in0=iota_rep[:],
                                scalar1=dcol[:, sg:sg + 1], scalar2=None,
                                op0=ALU.is_equal,
                            )
                        if op_limit < 2:
                            hOut = spool.tile([128, HC], F16, tag="hOut")
                            nc.vector.tensor_copy(out=hOut[:], in_=G[:, 0, :])
                            h_sink(s, hOut, mval, pool, ppool)
                            continue
                        # v = ea (x) we + G + R accumulated on PE into PSUM
                        # (rank-1 matmul + two identity matmuls per subgroup);
                        # Act reads each PSUM pair directly: u = prelu(v)
                        # (Prelu: same fn as lrelu, but shares the act table
                        # set with Exp -> no table reloads)
                        u = pool.tile([128, NSG, HC], F16, tag="u")
                        q0 = 0
                        while q0 < NSG:
                            qn = min(2, NSG - q0)
                            accV = vpool.tile([128, 2, HC], F32, tag="accV")
                            for j in range(qn):
                                sg = q0 + j
                                nc.tensor.matmul(
                                    out=accV[:, j, :],
                                    lhsT=eaT_t[0:1, sg * 128:(sg + 1) * 128],
                                    rhs=we_rep[0:1, :], start=True, stop=False)
                                nc.tensor.matmul(
                                    out=accV[:, j, :], lhsT=ident16[:],
                                    rhs=G[:, sg, :], start=False, stop=False)
                                nc.tensor.matmul(
                                    out=accV[:, j, :], lhsT=ident8[:],
                                    rhs=R[:, sg, :], start=False, stop=True)
                            nc.scalar.activation(
                                out=u[:, q0:q0 + qn, :], in_=accV[:, 0:qn, :],
                                func=AF.Prelu, alpha=0.2)
                            q0 += qn

                        if t_udump is not None and L == 1 and s == 0:
                            nc.sync.dma_start(
                                out=t_udump[:, :],
                                in_=u[:].rearrange("p s c -> p (s c)"))
                            nc.sync.dma_start(
                                out=t_gdump[:, :],
                                in_=G[:].rearrange("p s c -> p (s c)"))
                        if op_limit < 3:
                            hOut = spool.tile([128, HC], F16, tag="hOut")
                            nc.vector.tensor_copy(out=hOut[:], in_=u[:, 0, :])
                            h_sink(s, hOut, mval, pool, ppool)
                            continue
                        # z = u*att, alpha = per-head sum
                        z = pool.tile([128, NSG, HC], F16, tag="z")
                        nc.vector.tensor_tensor(
                            out=z[:, :, :], in0=u[:, :, :], in1=attB[:],
                            op=ALU.mult
                        )
                        # per-head sums via binary fold tree
                        zf = pool.tile([128, NSG, 4, 32], F16, tag="zf")
                        z4 = z[:].rearrange("p s (h c) -> p s h c", h=4)
                        nc.vector.tensor_tensor(
                            out=zf[:, :, :, :], in0=z4[:, :, :, 0:32],
                            in1=z4[:, :, :, 32:64], op=ALU.add,
                        )
                        w = 16
                        while w >= 2:
                            nc.vector.tensor_tensor(
                                out=zf[:, :, :, 0:w], in0=zf[:, :, :, 0:w],
                                in1=zf[:, :, :, w:2 * w], op=ALU.add,
                            )
                            w //= 2
                        alpha = spool.tile([128, 4 * NSG], F32, tag="alpha")
                        nc.vector.tensor_tensor(
                            out=alpha[:].rearrange("p (s h o) -> p s h o",
                                                   h=4, o=1),
                            in0=zf[:, :, :, 0:1], in1=zf[:, :, :, 1:2],
                            op=ALU.add,
                        )
                        # exB = exp(alpha) broadcast-materialized over the C
                        # dim in one Act instruction: m2 then hits DVE 2x mode
                        exB = pool.tile([128, NSG, 4, C], F16, tag="exB")
                        nc.scalar.activation(
                            out=exB[:, :, :, :],
                            in_=alpha[:].rearrange(
                                "p (s h o) -> p s h o", h=4, o=1
                            ).broadcast_to((128, NSG, 4, C)),
                            func=AF.Exp,
                        )

                        if op_limit < 4:
                            hOut = spool.tile([128, HC], F16, tag="hOut")
                            nc.vector.tensor_copy(out=hOut[:], in_=u[:, 0, :])
                            nc.vector.tensor_scalar(
                                out=hOut[:, 0:4], in0=exB[:, 0, 0:4, 0],
                                scalar1=1.0,
                                scalar2=None, op0=ALU.mult)
                            h_sink(s, hOut, mval, pool, ppool)
                            continue
                        if t_adump is not None and L == 1 and s == 0:
                            nc.sync.dma_start(out=t_adump[:, :], in_=alpha[:])
                            nc.sync.dma_start(
                                out=t_edump[:, :],
                                in_=exB[:].rearrange("p s h c -> p (s h c)"))
                        # m2 = ex * G (softmax-weighted source messages;
                        # out = sum a*xl[src] directly, no xr/we correction)
                        m2 = pool.tile([128, NSG, HC], F16, tag="m2")
                        nc.vector.tensor_tensor(
                            out=m2[:],
                            in0=G[:].rearrange("p s (h c) -> p s h c", h=4),
                            in1=exB[:, :, :, :],
                            op=ALU.mult,
                        )
                        if t_mdump is not None and L == 1 and s == 0:
                            nc.sync.dma_start(
                                out=t_mdump[:, :],
                                in_=m2[:].rearrange("p s c -> p (s c)"))
                        accM = ppool.tile([128, HC], F32, tag="accM")
                        accE = ppool.tile([128, 4], F32, tag="accE")
                        for sg in range(NSG):
                            nc.tensor.matmul(out=accM[:],
                                             lhsT=S[:, sg, :],
                                             rhs=m2[:, sg, :], start=(sg == 0),
                                             stop=(sg == NSG - 1))
                            nc.tensor.matmul(out=accE[:],
                                             lhsT=S[:, sg, :],
                                             rhs=exB[:, sg, :, 0],
                                             start=(sg == 0), stop=(sg == NSG - 1))

                        if op_limit < 5:
                            hOut = spool.tile([128, HC], F16, tag="hOut")
                            nc.vector.tensor_copy(out=hOut[:], in_=accM[:])
                            h_sink(s, hOut, mval, pool, ppool)
                            continue
                        if t_acdump is not None and L == 1 and s == 0:
                            acs = spool.tile([128, HC + 4], F32, tag="acdbg")
                            nc.vector.tensor_copy(out=acs[:, 0:HC], in_=accM[:])
                            nc.vector.tensor_copy(out=acs[:, HC:HC + 4],
                                                  in_=accE[:])
                            nc.sync.dma_start(out=t_acdump[:, :], in_=acs[:])
                        # flush: h = relu(accM/den + bias)
                        den = spool.tile([128, 4], F32, tag="den")
                        nc.vector.tensor_scalar(
                            out=den[:], in0=accE[:], scalar1=1e-30,
                            scalar2=None, op0=ALU.add,
                        )
                        rden = spool.tile([128, 4], F32, tag="rden")
                        nc.vector.reciprocal(out=rden[:], in_=den[:])
                        hT = spool.tile([128, HC], F16, tag="hT")
                        for hh in range(4):
                            blks = slice(hh * C, (hh + 1) * C)
                            nc.vector.scalar_tensor_tensor(
                                out=hT[:, blks], in0=accM[:, blks],
                                scalar=rden[:, hh:hh + 1], in1=bias_rep[:, blks],
                                op0=ALU.mult, op1=ALU.add,
                            )
                        hOut = spool.tile([128, HC], F16, tag="hOut")
                        nc.scalar.activation(out=hOut[:], in_=hT[:], func=AF.Relu)
                        h_sink(s, hOut, mval, pool, ppool)

            # layer 1: sink writes h1 to DRAM
            def h1_sink(s, hOut, mval, pool, ppool):
                nc.sync.dma_start(out=t_h1[s * 128:(s + 1) * 128, :], in_=hOut[:])
                if t_h1o is not None:
                    nc.sync.dma_start(out=t_h1o[s * 128:(s + 1) * 128, :],
                                      in_=hOut[:])

            if phase_limit >= 2:
                gat_layer(1, t_xl1, t_xr1, h1_sink)

            # --------------------------------------------------------------
            # Phase 4: xl2 from h1 -> AllGather kickoff -> xr2
            # (xr2 compute overlaps the collective; layer 2's R-gathers are
            # the only consumers of xr2 and start after the table arrives)
            # --------------------------------------------------------------
            with tc.tile_pool(name="p4", bufs=3) as pool, \
                 tc.tile_pool(name="p4ps", bufs=2, space="PSUM") as ppool:
                def xw2(s, nm0, sink0, nm1, sink1):
                    h1T0 = pool.tile([128, 128], F16, tag="h1T0")
                    h1T1 = pool.tile([128, 128], F16, tag="h1T1")
                    nc.sync.dma_start(
                        out=h1T0[:], in_=t_h1[s * 128:(s + 1) * 128, 0:128],
                        transpose=True,
                    )
                    nc.sync.dma_start(
                        out=h1T1[:], in_=t_h1[s * 128:(s + 1) * 128, 128:256],
                        transpose=True,
                    )
                    for nm, sink, dt in ((nm0, sink0, F16), (nm1, sink1, F8)):
                        wa, wb, wc = w2_tiles[nm]
                        ps = ppool.tile([128, HC], F32, tag="ps")
                        nc.tensor.matmul(out=ps[:], lhsT=h1T0[:], rhs=wa[:],
                                         start=True, stop=False)
                        nc.tensor.matmul(out=ps[:], lhsT=h1T1[:], rhs=wb[:],
                                         start=False, stop=False)
                        nc.tensor.matmul(out=ps[:], lhsT=ones_col[:],
                                         rhs=wc[:], start=False, stop=True)
                        xs = pool.tile([128, HC], dt, tag=f"xs{mybir.dt.size(dt)}")
                        nc.vector.tensor_copy(out=xs[:], in_=ps[:])
                        nc.sync.dma_start(out=sink[s * 128:(s + 1) * 128, :],
                                          in_=xs[:])

                for s in range(nspans if phase_limit >= 3 else 0):
                    xw2(s, "wl_aug", t_xl2_in, "wr_aug", t_xr2)

                # ----------------------------------------------------------
                # Phase 5: AllGather xl2
                # ----------------------------------------------------------
                if phase_limit >= 4:
                    nc.gpsimd.collective_compute(
                        "AllGather",
                        ALU.bypass,
                        replica_groups=[list(range(NCORES))],
                        ins=[t_xl2_in.ap().opt()],
                        outs=[t_xl2.ap().opt()],
                    )

            # --------------------------------------------------------------
            # Phase 6: GAT layer 2 with fused pooling
            # --------------------------------------------------------------
            if phase_limit < 5:
                with tc.tile_pool(name="dummyout", bufs=1) as dpool:
                    dz = dpool.tile([4, 64], F32)
                    nc.vector.memset(dz[:], 0.0)
                    nc.sync.dma_start(out=t_out[:], in_=dz[:])
                return
            with tc.tile_pool(name="gpool_ps", bufs=1, space="PSUM") as gpool_ps:
                gpsum = gpool_ps.tile([4, HC], F32)

                def h2_sink(s, hOut, mval, pool, ppool):
                    nc.tensor.matmul(out=gpsum[:], lhsT=mval[:, MV_G0:MV_G1],
                                     rhs=hOut[:],
                                     start=(s == 0), stop=(s == nspans - 1))

                gat_layer(2, t_xl2, t_xr2, h2_sink)

                # ----------------------------------------------------------
                # Phase 7: pooling -> MLP -> out
                # ----------------------------------------------------------
                with tc.tile_pool(name="mlp", bufs=1) as pool, \
                     tc.tile_pool(name="mlp_ps", bufs=2, space="PSUM") as ppool:
                    g = pool.tile([4, HC], F32)
                    nc.vector.tensor_scalar(out=g[:], in0=gpsum[:],
                                            scalar1=icnt[:, 0:1],
                                            scalar2=None, op0=ALU.mult)
                    # gT via PE transpose (two halves)
                    gT = pool.tile([128, 8], F32)
                    for half in range(2):
                        tp = ppool.tile([128, 128], F32, tag="tp")
                        nc.tensor.transpose(
                            out=tp[:, 0:4],
                            in_=g[:, half * 128:(half + 1) * 128],
                            identity=ident[0:4, 0:4],
                        )
                        nc.vector.tensor_copy(out=gT[:, half * 4:half * 4 + 4],
                                              in_=tp[:, 0:4])
                    z1p = ppool.tile([4, 128], F32, tag="z1p")
                    nc.tensor.matmul(out=z1p[:], lhsT=gT[:, 0:4], rhs=p1a[:],
                                     start=True, stop=False)
                    nc.tensor.matmul(out=z1p[:], lhsT=gT[:, 4:8], rhs=p1b[:],
                                     start=False, stop=False)
                    nc.tensor.matmul(out=z1p[:], lhsT=onesg[:], rhs=p1c[:],
                                     start=False, stop=True)
                    z1 = pool.tile([4, 128], F32)
                    nc.vector.tensor_copy(out=z1[:], in_=z1p[:])
                    # layernorm over free dim (128)
                    mu = pool.tile([4, 1], F32)
                    nc.vector.reduce_sum(out=mu[:], in_=z1[:], axis=AXX)
                    nc.vector.tensor_scalar(out=mu[:], in0=mu[:],
                                            scalar1=1.0 / 128,
                                            scalar2=None, op0=ALU.mult)
                    zc = pool.tile([4, 128], F32)
                    nc.vector.tensor_scalar(out=zc[:], in0=z1[:],
                                            scalar1=mu[:, 0:1],
                                            scalar2=None, op0=ALU.subtract)
                    sq = pool.tile([4, 128], F32)
                    nc.vector.tensor_tensor(out=sq[:], in0=zc[:], in1=zc[:],
                                            op=ALU.mult)
                    var = pool.tile([4, 1], F32)
                    nc.vector.reduce_sum(out=var[:], in_=sq[:], axis=AXX)
                    nc.vector.tensor_scalar(out=var[:], in0=var[:],
                                            scalar1=1.0 / 128,
                                            scalar2=1e-5, op0=ALU.mult,
                                            op1=ALU.add)
                    std = pool.tile([4, 1], F32)
                    nc.scalar.activation(out=std[:], in_=var[:], func=AF.Sqrt)
                    rstd = pool.tile([4, 1], F32)
                    nc.vector.reciprocal(out=rstd[:], in_=std[:])
                    zn = pool.tile([4, 128], F32)
                    nc.vector.tensor_scalar(out=zn[:], in0=zc[:],
                                            scalar1=rstd[:, 0:1],
                                            scalar2=None, op0=ALU.mult)
                    nc.vector.tensor_tensor(out=zn[:], in0=zn[:], in1=lng[:],
                                            op=ALU.mult)
                    nc.vector.tensor_tensor(out=zn[:], in0=zn[:], in1=lnb[:],
                                            op=ALU.add)
                    nc.scalar.activation(out=zn[:], in_=zn[:], func=AF.Relu)
                    # z2 = relu(zn @ p2 + b2)
                    znT = pool.tile([128, 4], F32)
                    tp2 = ppool.tile([128, 128], F32, tag="tp")
                    nc.tensor.transpose(out=tp2[:, 0:4], in_=zn[:],
                                        identity=ident[0:4, 0:4])
                    nc.vector.tensor_copy(out=znT[:], in_=tp2[:, 0:4])
                    z2p = ppool.tile([4, 64], F32, tag="z2p")
                    nc.tensor.matmul(out=z2p[:], lhsT=znT[:], rhs=p2a[:],
                                     start=True, stop=False)
                    nc.tensor.matmul(out=z2p[:], lhsT=onesg[:], rhs=p2c[:],
                                     start=False, stop=True)
                    zout = pool.tile([4, 64], F32)
                    nc.scalar.activation(out=zout[:], in_=z2p[:], func=AF.Relu)
                    nc.sync.dma_start(out=t_out[:], in_=zout[:])

        for _rep in range(reps):
            build_rep()

    nc.finalize()
    # Tile assigns SWDGE completion-sem lanes (8) round-robin over Pool DMA
    # insts in SCHEDULED order, and each lane must stick to one queue.  The
    # scheduler reorders gathers across spans, so rewrite queue_num in
    # scheduled order: lane i%8 <-> queue i%nqueues (nqueues | 8).
    i = 0
    for blk in nc.m.functions[0].blocks:
        for inst in blk.instructions:
            if inst.engine == mybir.EngineType.Pool and isinstance(
                inst, mybir.InstDMAGatherAnt
            ):
                inst.queue_num = i % nqueues
                i += 1
    return nc


# ----------------------------------------------------------------------------
# Entry point
# ----------------------------------------------------------------------------

def _pack_inputs(inp, cores, packs, nspans, rows_per_core, rows_total, x_aug_T):
    f16 = np.float16
    f32 = np.float32
    # shared (replicated) blocks
    wpack = np.zeros((WP_ROWS, HC), dtype=f16)

    def aug(w, b):
        return np.concatenate(
            [np.asarray(w, f32), np.asarray(b, f32)[None, :]], 0
        ).astype(f16)

    wpack[WP_WL1:WP_WL1 + 65] = aug(inp["g1_wl"], inp["g1_bl"])
    wpack[WP_WR1:WP_WR1 + 65] = aug(inp["g1_wr"], inp["g1_br"])
    wpack[WP_WL2:WP_WL2 + 257] = aug(inp["g2_wl"], inp["g2_bl"])
    wpack[WP_WR2:WP_WR2 + 257] = aug(inp["g2_wr"], inp["g2_br"])
    for L, (r_att, r_we, r_bias) in (
        (1, (WP_ATT1, WP_WE1, WP_BIAS1)),
        (2, (WP_ATT2, WP_WE2, WP_BIAS2)),
    ):
        wpack[r_att:r_att + 128] = np.broadcast_to(
            np.asarray(inp[f"g{L}_att"], f32).reshape(1, HC), (128, HC)
        ).astype(f16)
        wpack[r_we:r_we + 128] = np.broadcast_to(
            np.asarray(inp[f"g{L}_we"], f32).reshape(1, HC), (128, HC)
        ).astype(f16)
        wpack[r_bias:r_bias + 128] = np.broadcast_to(
            np.asarray(inp[f"g{L}_bias"], f32).reshape(1, HC), (128, HC)
        ).astype(f16)
    wpack[WP_IOTA:WP_IOTA + 128, 0:128] = np.broadcast_to(
        np.arange(128, dtype=f16)[None, :], (128, 128)
    )

    mlp = np.zeros((MP_ROWS, 128), dtype=f32)
    mlp[MP_P1:MP_P1 + 257] = np.concatenate(
        [np.asarray(inp["p1_w"], f32), np.asarray(inp["p1_b"], f32)[None, :]], 0
    )
    mlp[MP_LNG:MP_LNG + 4] = np.asarray(inp["ln_g"], f32)[None, :]
    mlp[MP_LNB:MP_LNB + 4] = np.asarray(inp["ln_b"], f32)[None, :]
    mlp[MP_P2:MP_P2 + 129, 0:64] = np.concatenate(
        [np.asarray(inp["p2_w"], f32), np.asarray(inp["p2_b"], f32)[None, :]], 0
    )

    enc_aug = np.concatenate(
        [np.asarray(inp["enc_w"], f32), np.asarray(inp["enc_b"], f32)[None, :]], 0
    )  # [5, 64]

    in_maps = []
    for k in range(NCORES):
        p = packs[k]
        feat = np.zeros((5, rows_total + rows_per_core + 64), dtype=f32)
        feat[:, 0:rows_total] = x_aug_T
        feat[:, rows_total:rows_total + rows_per_core] = x_aug_T[:, p["own_cols"]]
        feat[:, rows_total + rows_per_core:] = enc_aug
        mlp_k = mlp.copy()
        mlp_k[MP_CNT:MP_CNT + 4, 0] = p["inv_cnt"]
        in_maps.append({
            "feat": feat,
            "wpack": wpack,
            "meta_idx": p["meta_idx"],
            "meta_val": p["meta_val"].view(np.float16),
            "eaT": p["ea_T"],
            "mlp": mlp_k,
        })
    return in_maps


def kernel(**inputs):
    cores, packs, nspans, rows_per_core, rows_total, x_aug_T, node_row = _host_prep(
        inputs
    )
    key = (nspans, rows_total)
    if key not in _PROGRAM_CACHE:
        _PROGRAM_CACHE[key] = _build_program(nspans, rows_total)
    nc = _PROGRAM_CACHE[key]
    in_maps = _pack_inputs(
        inputs, cores, packs, nspans, rows_per_core, rows_total, x_aug_T
    )
    res = run_bass_kernel_spmd(nc, in_maps, core_ids=list(range(NCORES)))
    out = np.concatenate([res.results[k]["out"] for k in range(NCORES)], axis=0)
    return out.astype(np.float32)


if __name__ == "__main__":
    data = dict(np.load("/root/problem/inputs_cache.npz"))
    out = kernel(**data)
    exp = np.load("/root/problem/expected_np.npy")
    rel = np.linalg.norm(out - exp) / np.linalg.norm(exp)
    print("rel err:", rel)



# revision 35
# speedup vs baseline: 1.2785x; 1.0557x over previous
"""Trainium2 Bass kernel for nn_GATv2Base (gnn_message_passing).

Contract: kernel(**inputs) takes FULL unsharded inputs (same keys as
reference.setup_inputs()) and returns the FULL [32, 64] float32 output.

Sharding: 32 graphs -> 8 cores (4 graphs each, contiguous node ranges since
`batch` is sorted).  Edges (plus self-loops) are routed to the core owning
their dst node, sorted by dst, and packed into "spans" (<=127-node dst window,
2304 edge slots = 18 subgroups of 128 edges).  Node features live in a
span-major padded global layout so every per-span device address is static.
Layer 1 runs fully local (xl1 table computed replicated from x); between
layers one fp16 AllGather shares the xl2 table; the pooled per-graph MLP is
computed on the owning core.  Only the [4, 64] per-core outputs return to the
host.

All per-core inputs are packed into 5 device tensors (feat/wpack/meta_idx/
meta_val/mlp) to minimize per-call argument-marshalling overhead on the host
runtime.  _build_program(reps=N) unrolls the whole computation N times inside
one program; timing two variants isolates true device execution time from
dispatch overhead.
"""

import os
import sys

import numpy as np

for _p in ("/opt/trn_rl_repo", "/root/.axon_site/_ro/trn_rl_repo"):
    if os.path.isdir(_p) and _p not in sys.path:
        sys.path.insert(0, _p)

import concourse.bass as bass
import concourse.bacc as bacc
import concourse.mybir as mybir
import concourse.tile as tile
from concourse.bass_utils import run_bass_kernel_spmd

F32 = mybir.dt.float32
F16 = mybir.dt.float16
F8 = mybir.dt.float8e4
I16 = mybir.dt.int16
I32 = mybir.dt.int32
AF = mybir.ActivationFunctionType
ALU = mybir.AluOpType
AXX = mybir.AxisListType.X

N, E, H, C, NG = 50000, 800000, 4, 64, 32
HC = H * C
NCORES = 8
SLOT_H = 1152            # edge slots per table-half region (9 subgroups)
SPAN_EDGES = 2 * SLOT_H  # 2304 edge slots per span (18 subgroups of 128)
NSG = SPAN_EDGES // 128  # 18
NSG_H = SLOT_H // 128    # 9
SPAN_DST = 127           # dst window per span; slot 127 = pad marker

# meta_idx layout (i16 cols): src_idx_a 0:72, src_idx_b 72:144, xr_idx 144:288
MI_A0, MI_A1 = 0, SLOT_H // 16
MI_B0, MI_B1 = MI_A1, 2 * (SLOT_H // 16)
MI_X0, MI_X1 = MI_B1, MI_B1 + SPAN_EDGES // 16
# meta_val layout (f16 cols): dcol 0:18, eac 18:36, gmask 36:40
MV_D0, MV_D1 = 0, NSG
MV_E0, MV_E1 = NSG, 2 * NSG
MV_G0, MV_G1 = 2 * NSG, 2 * NSG + 4
MV_W = MV_G1
# wpack rows (f16, width 256)
WP_WL1, WP_WR1 = 0, 65
WP_ATT1, WP_WE1, WP_BIAS1 = 130, 258, 386
WP_WL2, WP_WR2 = 514, 771
WP_ATT2, WP_WE2, WP_BIAS2 = 1028, 1156, 1284
WP_IOTA = 1412
WP_ROWS = 1540
# mlp rows (f32, width 128)
MP_P1, MP_LNG, MP_LNB, MP_P2, MP_CNT = 0, 257, 261, 265, 394
MP_ROWS = 398


# ----------------------------------------------------------------------------
# Host-side sharding / packing
# ----------------------------------------------------------------------------

def _host_prep(inp):
    x = np.asarray(inp["x"], dtype=np.float32)
    ei = np.asarray(inp["edge_index"], dtype=np.int32)
    ea_full = np.asarray(inp["edge_attr"], dtype=np.float32)[:, 0]
    batch = np.asarray(inp["batch"], dtype=np.int32)

    src0, dst0 = ei[0], ei[1]
    deg = np.maximum(np.bincount(dst0, minlength=N).astype(np.float64), 1.0)
    loop_attr = (
        np.bincount(dst0, weights=ea_full.astype(np.float64), minlength=N) / deg
    ).astype(np.float32)
    src = np.concatenate([src0, np.arange(N, dtype=np.int32)])
    dst = np.concatenate([dst0, np.arange(N, dtype=np.int32)])
    eattr = np.concatenate([ea_full, loop_attr]).astype(np.float32)

    gcounts = np.bincount(batch, minlength=NG)
    gstart = np.concatenate([[0], np.cumsum(gcounts)])
    core_n0 = np.array([gstart[4 * k] for k in range(NCORES)] + [N], dtype=np.int64)

    order = np.argsort(dst, kind="stable")
    src, dst, eattr = src[order], dst[order], eattr[order]
    edge_lo = np.searchsorted(dst, core_n0[:-1], "left")
    edge_hi = np.searchsorted(dst, core_n0[1:], "left")

    # src owner core (cores 0-3 -> table half A, 4-7 -> half B); stable
    # under span-count changes so it can drive packing.
    src_owner = np.searchsorted(core_n0[1:], src, "right")
    src_in_a = src_owner < (NCORES // 2)

    cores = []
    for k in range(NCORES):
        n0, n1 = int(core_n0[k]), int(core_n0[k + 1])
        s, e = int(edge_lo[k]), int(edge_hi[k])
        cd = dst[s:e]
        ca = src_in_a[s:e]
        nlocal = n1 - n0
        node_edge_start = np.searchsorted(cd, n0 + np.arange(nlocal + 1))
        cumA = np.concatenate([[0], np.cumsum(ca)])  # over edges
        spans = []
        b = 0
        while b < nlocal:
            bend = b
            while bend < nlocal and (bend - b) < SPAN_DST:
                e0, e1 = node_edge_start[b], node_edge_start[bend + 1]
                nA = cumA[e1] - cumA[e0]
                nB = (e1 - e0) - nA
                if nA > SLOT_H or nB > SLOT_H:
                    break
                bend += 1
            assert bend > b, "single node exceeds span edge capacity"
            spans.append(
                (b, bend - b, int(node_edge_start[b]), int(node_edge_start[bend]))
            )
            b = bend
        cores.append(
            dict(n0=n0, n1=n1, spans=spans, src=src[s:e], dst=cd, ea=eattr[s:e],
                 in_a=ca)
        )

    nspans = max(len(c["spans"]) for c in cores)
    rows_per_core = nspans * 128
    rows_total = NCORES * rows_per_core

    # global padded row per node
    node_row = np.zeros(N, dtype=np.int64)
    for k, c in enumerate(cores):
        for si, (b, nb, _, _) in enumerate(c["spans"]):
            nodes = np.arange(c["n0"] + b, c["n0"] + b + nb)
            node_row[nodes] = k * rows_per_core + si * 128 + (nodes - c["n0"] - b)

    # x padded, transposed, with ones row (for encoder rhs)
    x_pad = np.zeros((rows_total, 4), dtype=np.float32)
    x_pad[node_row] = x
    x_aug_T = np.concatenate(
        [x_pad.T, np.ones((1, rows_total), dtype=np.float32)], axis=0
    )  # [5, R]

    packs = []
    half_rows = rows_total // 2
    assert half_rows <= 32767, f"table half {half_rows} exceeds int16 index range"

    def wrap_idx16(vals):
        # vals: [SLOT] int -> wrapped [128, SLOT//16] int16 (16-part wrap,
        # replicated over the 8 q7 core groups)
        slot = len(vals)
        base = np.zeros((16, slot // 16), dtype=np.int16)
        i = np.arange(slot)
        base[i % 16, i // 16] = vals.astype(np.int16)
        return np.tile(base, (8, 1))

    for k, c in enumerate(cores):
        meta_idx = np.zeros((nspans, 128, MI_X1), dtype=np.int16)
        meta_val = np.zeros((nspans, 128, MV_W), dtype=np.float16)
        ea_T = np.zeros((nspans, SPAN_EDGES), dtype=np.float16)
        meta_val[:, :, MV_D0:MV_D1] = np.float16(127.0)
        for si, (b, nb, e0, e1) in enumerate(c["spans"]):
            ina = c["in_a"][e0:e1]
            esrc = node_row[c["src"][e0:e1]]
            edrel = (c["dst"][e0:e1] - c["n0"] - b).astype(np.int64)
            eea = c["ea"][e0:e1]
            # slots: A edges first (in region [0, SLOT_H)), then B edges at
            # [SLOT_H, 2*SLOT_H); pads keep idx 0 / drel 127 / ea 0
            ia = np.where(ina)[0]
            ib = np.where(~ina)[0]
            slots = np.empty(len(ina), dtype=np.int64)
            slots[ia] = np.arange(len(ia))
            slots[ib] = SLOT_H + np.arange(len(ib))
            av = np.zeros(SLOT_H, dtype=np.int64)
            av[:len(ia)] = esrc[ia]
            bv = np.zeros(SLOT_H, dtype=np.int64)
            bv[:len(ib)] = esrc[ib] - half_rows
            meta_idx[si, :, MI_A0:MI_A1] = wrap_idx16(av)
            meta_idx[si, :, MI_B0:MI_B1] = wrap_idx16(bv)
            xv = np.full(SPAN_EDGES, si * 128 + 127, dtype=np.int64)
            xv[slots] = si * 128 + edrel
            meta_idx[si, :, MI_X0:MI_X1] = wrap_idx16(xv)
            p, sg = slots % 128, slots // 128
            meta_val[si, p, MV_D0 + sg] = edrel.astype(np.float16)
            meta_val[si, p, MV_E0 + sg] = eea.astype(np.float16)
            ea_T[si, slots] = eea.astype(np.float16)
            nodes = np.arange(c["n0"] + b, c["n0"] + b + nb)
            gl = batch[nodes] - 4 * k
            meta_val[si, np.arange(nb), MV_G0 + gl] = np.float16(1.0)
        inv_cnt = np.zeros((4,), dtype=np.float32)
        for gg in range(4):
            cnt = max(int(gcounts[4 * k + gg]), 1)
            inv_cnt[gg] = 1.0 / cnt
        packs.append(
            dict(
                meta_idx=meta_idx,
                meta_val=meta_val,
                ea_T=ea_T,
                inv_cnt=inv_cnt,
                own_cols=np.arange(
                    k * rows_per_core, (k + 1) * rows_per_core, dtype=np.int64
                ),
            )
        )
    return cores, packs, nspans, rows_per_core, rows_total, x_aug_T, node_row


# ----------------------------------------------------------------------------
# Device program
# ----------------------------------------------------------------------------

_PROGRAM_CACHE = {}


def _build_program(nspans, rows_total, reps=1, phase_limit=5, op_limit=9,
                   single_packet=False, nqueues=4, skip_r=False,
                   debug_h1=False):
    rows_per_core = nspans * 128
    nblocks = rows_total // 128

    nc = bacc.Bacc(num_swdge_queues=nqueues)
    tcx = tile.TileContext(nc)

    t_feat = nc.dram_tensor(
        "feat", [5, rows_total + rows_per_core + 64], F32, kind="ExternalInput"
    )
    t_wpack = nc.dram_tensor("wpack", [WP_ROWS, HC], F16, kind="ExternalInput")
    t_midx = nc.dram_tensor(
        "meta_idx", [nspans, 128, MI_X1], I16, kind="ExternalInput"
    )
    t_mval = nc.dram_tensor(
        "meta_val", [nspans, 128, MV_W], F16, kind="ExternalInput"
    )
    t_eaT = nc.dram_tensor("eaT", [nspans, SPAN_EDGES], F16, kind="ExternalInput")
    t_mlp = nc.dram_tensor("mlp", [MP_ROWS, 128], F32, kind="ExternalInput")
    t_out = nc.dram_tensor("out", [4, 64], F32, kind="ExternalOutput")

    # ---- internal DRAM ----
    t_xl1_own = nc.dram_tensor("xl1_own_cc", [rows_per_core, HC], F16)
    t_xl1 = nc.dram_tensor("xl1_tbl", [rows_total, HC], F16, addr_space="Shared")
    t_xr1 = nc.dram_tensor("xr1_own", [rows_per_core, HC], F8)
    t_h1 = nc.dram_tensor("h1_own", [rows_per_core, HC], F16)
    t_h1o = (nc.dram_tensor("h1dump", [rows_per_core, HC], F16,
                            kind="ExternalOutput") if debug_h1 else None)
    t_udump = (nc.dram_tensor("udump", [128, NSG * HC], F16,
                              kind="ExternalOutput") if debug_h1 else None)
    t_gdump = (nc.dram_tensor("gdump", [128, NSG * HC], F16,
                              kind="ExternalOutput") if debug_h1 else None)
    t_adump = (nc.dram_tensor("adump", [128, 4 * NSG], F32,
                              kind="ExternalOutput") if debug_h1 else None)
    t_edump = (nc.dram_tensor("edump", [128, NSG * HC], F16,
                              kind="ExternalOutput") if debug_h1 else None)
    t_acdump = (nc.dram_tensor("acdump", [128, HC + 4], F32,
                               kind="ExternalOutput") if debug_h1 else None)
    t_mdump = (nc.dram_tensor("mdump", [128, NSG * HC], F16,
                              kind="ExternalOutput") if debug_h1 else None)
    t_xr2 = nc.dram_tensor("xr2_own", [rows_per_core, HC], F8)
    t_xl2_in = nc.dram_tensor("xl2_own_cc", [rows_per_core, HC], F16)
    t_xl2 = nc.dram_tensor("xl2_tbl", [rows_total, HC], F16, addr_space="Shared")

    from contextlib import ExitStack
    with tcx as tc, ExitStack() as es:
        # ------------------------------------------------------------------
        # constants in SBUF (loaded once, reused by every rep)
        # ------------------------------------------------------------------
        cpool = es.enter_context(tc.tile_pool(name="consts", bufs=1))
        enc_aug = cpool.tile([5, 64], F32)
        nc.sync.dma_start(
            out=enc_aug[:],
            in_=t_feat[:, rows_total + rows_per_core:rows_total + rows_per_core + 64],
        )
        iota_rep = cpool.tile([128, 128], F16)
        nc.sync.dma_start(out=iota_rep[:], in_=t_wpack[WP_IOTA:WP_IOTA + 128, 0:128])
        reps_t = {}
        for L, (r_att, r_we, r_bias) in (
            (1, (WP_ATT1, WP_WE1, WP_BIAS1)),
            (2, (WP_ATT2, WP_WE2, WP_BIAS2)),
        ):
            for nm, r0 in (("att_row", r_att), ("we_row", r_we), ("bias_row", r_bias)):
                rep = cpool.tile([128, HC], F16, tag=f"rep{L}{nm}")
                nc.sync.dma_start(out=rep[:], in_=t_wpack[r0:r0 + 128, :])
                reps_t[(L, nm)] = rep
        ones_col = cpool.tile([1, 128], F16)
        nc.vector.memset(ones_col[:], 1.0)
        ones_row = cpool.tile([1, 512], F16)
        nc.vector.memset(ones_row[:], 1.0)
        from concourse.masks import make_identity
        ident16 = cpool.tile([128, 128], F16)
        make_identity(nc, ident16[:])
        ident8 = cpool.tile([128, 128], F8)
        nc.vector.tensor_copy(out=ident8[:], in_=ident16[:])

        wpool = es.enter_context(tc.tile_pool(name="weights", bufs=1))
        wl1 = wpool.tile([65, HC], F16)
        wr1 = wpool.tile([65, HC], F16)
        nc.sync.dma_start(out=wl1[:], in_=t_wpack[WP_WL1:WP_WL1 + 65, :])
        nc.sync.dma_start(out=wr1[:], in_=t_wpack[WP_WR1:WP_WR1 + 65, :])
        w2_tiles = {}
        for nm, r0 in (("wl_aug", WP_WL2), ("wr_aug", WP_WR2)):
            a = wpool.tile([128, HC], F16, tag=f"{nm}a")
            b = wpool.tile([128, HC], F16, tag=f"{nm}b")
            cbias = wpool.tile([1, HC], F16, tag=f"{nm}c")
            nc.sync.dma_start(out=a[:], in_=t_wpack[r0:r0 + 128, :])
            nc.sync.dma_start(out=b[:], in_=t_wpack[r0 + 128:r0 + 256, :])
            nc.sync.dma_start(out=cbias[:], in_=t_wpack[r0 + 256:r0 + 257, :])
            w2_tiles[nm] = (a, b, cbias)
        # MLP constants
        mpool = es.enter_context(tc.tile_pool(name="mlpc", bufs=1))
        p1a = mpool.tile([128, 128], F32)
        p1b = mpool.tile([128, 128], F32)
        p1c = mpool.tile([1, 128], F32)
        nc.sync.dma_start(out=p1a[:], in_=t_mlp[MP_P1:MP_P1 + 128, :])
        nc.sync.dma_start(out=p1b[:], in_=t_mlp[MP_P1 + 128:MP_P1 + 256, :])
        nc.sync.dma_start(out=p1c[:], in_=t_mlp[MP_P1 + 256:MP_P1 + 257, :])
        p2a = mpool.tile([128, 64], F32)
        p2c = mpool.tile([1, 64], F32)
        nc.sync.dma_start(out=p2a[:], in_=t_mlp[MP_P2:MP_P2 + 128, 0:64])
        nc.sync.dma_start(out=p2c[:], in_=t_mlp[MP_P2 + 128:MP_P2 + 129, 0:64])
        lng = mpool.tile([4, 128], F32)
        nc.sync.dma_start(out=lng[:], in_=t_mlp[MP_LNG:MP_LNG + 4, :])
        lnb = mpool.tile([4, 128], F32)
        nc.sync.dma_start(out=lnb[:], in_=t_mlp[MP_LNB:MP_LNB + 4, :])
        icnt = mpool.tile([4, 1], F32)
        nc.sync.dma_start(out=icnt[:], in_=t_mlp[MP_CNT:MP_CNT + 4, 0:1])
        ident = mpool.tile([128, 128], F32)
        from concourse.masks import make_identity
        make_identity(nc, ident[:])
        onesg = mpool.tile([1, 4], F32)
        nc.vector.memset(onesg[:], 1.0)

        def encode4(pool, ppool, col0, ncols):
            """Encode ncols (<=512) padded nodes starting at feat col col0.
            Returns h0T4 [65, ncols] f16 (aug ones row included)."""
            xT = pool.tile([5, 512], F32, tag="xT")
            nc.sync.dma_start(out=xT[:, 0:ncols], in_=t_feat[:, col0:col0 + ncols])
            h0p = ppool.tile([64, 512], F32, tag="h0ps")
            nc.tensor.matmul(out=h0p[:, 0:ncols], lhsT=enc_aug[:],
                             rhs=xT[:, 0:ncols], start=True, stop=True)
            h0T = pool.tile([65, 512], F16, tag="h0T")
            nc.scalar.activation(out=h0T[0:64, 0:ncols], in_=h0p[:, 0:ncols],
                                 func=AF.Relu)
            nc.vector.tensor_copy(out=h0T[64:65, 0:ncols],
                                  in_=ones_row[:, 0:ncols])
            return h0T

        def xw_blocks(pool, ppool, h0T, w, nblk, sink_ap, dt=F16):
            """nblk xl/xr matmuls from h0T slices; one batched DMA to DRAM."""
            xls = pool.tile([128, 4, HC], dt, tag=f"xls{mybir.dt.size(dt)}")
            for j in range(nblk):
                xlp = ppool.tile([128, HC], F32, tag="xlps")
                nc.tensor.matmul(out=xlp[:], lhsT=h0T[:, j * 128:(j + 1) * 128],
                                 rhs=w[:], start=True, stop=True)
                if j % 2 == 0:
                    nc.vector.tensor_copy(out=xls[:, j, :], in_=xlp[:])
                else:
                    nc.scalar.copy(out=xls[:, j, :], in_=xlp[:])
            nc.sync.dma_start(
                out=sink_ap.rearrange("(b p) c -> p b c", p=128),
                in_=xls[:, 0:nblk, :],
            )

        def build_rep():
            # --------------------------------------------------------------
            # Phase 1: encoder + xl1 for ALL rows (4 blocks per DMA batch)
            # --------------------------------------------------------------
            with tc.tile_pool(name="p1", bufs=3) as pool, \
                 tc.tile_pool(name="p1ps", bufs=2, space="PSUM") as ppool:
                # encode ONLY own rows once; xl1-own and xr1 share the encode,
                # then one AllGather replicates the xl1 table (replaces the
                # 8x-redundant full-table compute)
                ns4 = (nspans + 3) // 4 if phase_limit >= 1 else 0
                for s4 in range(ns4):
                    nblk = min(4, nspans - s4 * 4)
                    h0T = encode4(pool, ppool, rows_total + s4 * 512, nblk * 128)
                    xw_blocks(pool, ppool, h0T, wl1, nblk,
                              t_xl1_own[s4 * 512:s4 * 512 + nblk * 128, :])
                    xw_blocks(pool, ppool, h0T, wr1, nblk,
                              t_xr1[s4 * 512:s4 * 512 + nblk * 128, :], dt=F8)
                if phase_limit >= 1:
                    nc.gpsimd.collective_compute(
                        "AllGather",
                        ALU.bypass,
                        replica_groups=[list(range(NCORES))],
                        ins=[t_xl1_own.ap().opt()],
                        outs=[t_xl1.ap().opt()],
                    )

            # --------------------------------------------------------------
            # GAT span loop (shared for both layers)
            # --------------------------------------------------------------
            def gat_layer(L, xl_tbl, xr_tbl, h_sink):
                """h_sink(s, htile, mval): consume flush output [128, HC] f16."""
                att_rep = reps_t[(L, "att_row")]
                we_rep = reps_t[(L, "we_row")]
                bias_rep = reps_t[(L, "bias_row")]
                with tc.tile_pool(name=f"g{L}", bufs=2) as pool, \
                     tc.tile_pool(name=f"g{L}c", bufs=1) as lpool, \
                     tc.tile_pool(name=f"g{L}b", bufs=3) as spool, \
                     tc.tile_pool(name=f"g{L}g", bufs=3) as gpool, \
                     tc.tile_pool(name=f"g{L}v", bufs=2, space="PSUM") as vpool, \
                     tc.tile_pool(name=f"g{L}ps", bufs=2, space="PSUM") as ppool:
                    half_rows = rows_total // 2
                    # att row materialized across subgroups once per layer so
                    # the per-span z multiply runs as a plain contiguous TT
                    attB = lpool.tile([128, NSG, HC], F16)
                    nc.vector.tensor_copy(
                        out=attB[:],
                        in_=att_rep[:].rearrange(
                            "p (o c) -> p o c", o=1
                        ).broadcast_to((128, NSG, HC)),
                    )
                    for s in range(nspans):
                        midx = spool.tile([128, MI_X1], I16, tag="midx")
                        nc.sync.dma_start(out=midx[:], in_=t_midx[s, :, :])
                        mval = spool.tile([128, MV_W], F16, tag="mval")
                        nc.sync.dma_start(out=mval[:], in_=t_mval[s, :, :])
                        eaT_t = spool.tile([1, SPAN_EDGES], F16, tag="eaT")
                        nc.sync.dma_start(out=eaT_t[:], in_=t_eaT[s:s + 1, :])
                        dcol = spool.tile([128, NSG], F32, tag="dcolF")
                        nc.vector.tensor_copy(out=dcol[:],
                                              in_=mval[:, MV_D0:MV_D1])
                        # G = xl[src] (two half-table gathers), R = xr[dst]
                        # (per-core fp8 xr table gather via packed xr_idx;
                        # fp8 halves the random-read HBM bytes and R only
                        # feeds the attention logits)
                        G = gpool.tile([128, NSG, HC], F16, tag="G")
                        nc.gpsimd.dma_gather(
                            G[:, 0:NSG_H, :], xl_tbl[0:half_rows, :],
                            midx[:, MI_A0:MI_A1],
                            SLOT_H, SLOT_H, HC, single_packet=single_packet,
                            queue_num=0,
                        )
                        nc.gpsimd.dma_gather(
                            G[:, NSG_H:NSG, :], xl_tbl[half_rows:, :],
                            midx[:, MI_B0:MI_B1],
                            SLOT_H, SLOT_H, HC, single_packet=single_packet,
                            queue_num=1 % nqueues,
                        )
                        R = gpool.tile([128, NSG, HC], F8, tag="R")
                        nc.gpsimd.dma_gather(
                            R[:, 0:NSG_H, :], xr_tbl[:, :],
                            midx[:, MI_X0:MI_X0 + SLOT_H // 16],
                            SLOT_H, SLOT_H, HC,
                            single_packet=single_packet,
                            queue_num=2 % nqueues,
                        )
                        nc.gpsimd.dma_gather(
                            R[:, NSG_H:NSG, :], xr_tbl[:, :],
                            midx[:, MI_X0 + SLOT_H // 16:MI_X1],
                            SLOT_H, SLOT_H, HC,
                            single_packet=single_packet,
                            queue_num=3 % nqueues,
                        )
                        # dst one-hot S (for the segment-sum matmuls)
                        S = pool.tile([128, NSG, 128], F16, tag="S")
                        for sg in range(NSG):
                            nc.vector.tensor_scalar(
                                out=S[:, sg, :], in0=iota_rep[:],
                                scalar1=dcol[:, sg:sg + 1], scalar2=None,
                                op0=ALU.is_equal,
                            )
                        if op_limit < 2:
                            hOut = spool.tile([128, HC], F16, tag="hOut")
                            nc.vector.tensor_copy(out=hOut[:], in_=G[:, 0, :])
                            h_sink(s, hOut, mval, pool, ppool)
                            continue
                        # v = ea (x) we + G + R accumulated on PE into PSUM
                        # (rank-1 matmul + two identity matmuls per subgroup);
                        # Act reads each PSUM pair directly: u = prelu(v)
                        # (Prelu: same fn as lrelu, but shares the act table
                        # set with Exp -> no table reloads)
                        u = pool.tile([128, NSG, HC], F16, tag="u")
                        q0 = 0
                        while q0 < NSG:
                            qn = min(2, NSG - q0)
                            accV = vpool.tile([128, 2, HC], F32, tag="accV")
                            for j in range(qn):
                                sg = q0 + j
                                nc.tensor.matmul(
                                    out=accV[:, j, :],
                                    lhsT=eaT_t[0:1, sg * 128:(sg + 1) * 128],
                                    rhs=we_rep[0:1, :], start=True, stop=False)
                                nc.tensor.matmul(
                                    out=accV[:, j, :], lhsT=ident16[:],
                                    rhs=G[:, sg, :], start=False, stop=False)
                                nc.tensor.matmul(
                                    out=accV[:, j, :], lhsT=ident8[:],
                                    rhs=R[:, sg, :], start=False, stop=True)
                            nc.scalar.activation(
                                out=u[:, q0:q0 + qn, :], in_=accV[:, 0:qn, :],
                                func=AF.Prelu, alpha=0.2)
                            q0 += qn

                        if t_udump is not None and L == 1 and s == 0:
                            nc.sync.dma_start(
                                out=t_udump[:, :],
                                in_=u[:].rearrange("p s c -> p (s c)"))
                            nc.sync.dma_start(
                                out=t_gdump[:, :],
                                in_=G[:].rearrange("p s c -> p (s c)"))
                        if op_limit < 3:
                            hOut = spool.tile([128, HC], F16, tag="hOut")
                            nc.vector.tensor_copy(out=hOut[:], in_=u[:, 0, :])
                            h_sink(s, hOut, mval, pool, ppool)
                            continue
                        # z = u*att, alpha = per-head sum
                        z = pool.tile([128, NSG, HC], F16, tag="z")
                        nc.vector.tensor_tensor(
                            out=z[:, :, :], in0=u[:, :, :], in1=attB[:],
                            op=ALU.mult
                        )
                        # per-head sums via binary fold tree
                        zf = pool.tile([128, NSG, 4, 32], F16, tag="zf")
                        z4 = z[:].rearrange("p s (h c) -> p s h c", h=4)
                        nc.vector.tensor_tensor(
                            out=zf[:, :, :, :], in0=z4[:, :, :, 0:32],
                            in1=z4[:, :, :, 32:64], op=ALU.add,
                        )
                        w = 16
                        while w >= 2:
                            nc.vector.tensor_tensor(
                                out=zf[:, :, :, 0:w], in0=zf[:, :, :, 0:w],
                                in1=zf[:, :, :, w:2 * w], op=ALU.add,
                            )
                            w //= 2
                        alpha = spool.tile([128, 4 * NSG], F32, tag="alpha")
                        nc.vector.tensor_tensor(
                            out=alpha[:].rearrange("p (s h o) -> p s h o",
                                                   h=4, o=1),
                            in0=zf[:, :, :, 0:1], in1=zf[:, :, :, 1:2],
                            op=ALU.add,
                        )
                        # exB = exp(alpha) broadcast-materialized over the C
                        # dim in one Act instruction: m2 then hits DVE 2x mode
                        exB = pool.tile([128, NSG, 4, C], F16, tag="exB")
                        nc.scalar.activation(
                            out=exB[:, :, :, :],
                            in_=alpha[:].rearrange(
                                "p (s h o) -> p s h o", h=4, o=1
                            ).broadcast_to((128, NSG, 4, C)),
                            func=AF.Exp,
                        )

                        if op_limit < 4:
                            hOut = spool.tile([128, HC], F16, tag="hOut")
                            nc.vector.tensor_copy(out=hOut[:], in_=u[:, 0, :])
                            nc.vector.tensor_scalar(
                                out=hOut[:, 0:4], in0=exB[:, 0, 0:4, 0],
                                scalar1=1.0,
                                scalar2=None, op0=ALU.mult)
                            h_sink(s, hOut, mval, pool, ppool)
                            continue
                        if t_adump is not None and L == 1 and s == 0:
                            nc.sync.dma_start(out=t_adump[:, :], in_=alpha[:])
                            nc.sync.dma_start(
                                out=t_edump[:, :],
                                in_=exB[:].rearrange("p s h c -> p (s h c)"))
                        # m2 = ex * G (softmax-weighted source messages;
                        # out = sum a*xl[src] directly, no xr/we correction)
                        m2 = pool.tile([128, NSG, HC], F16, tag="m2")
                        nc.vector.tensor_tensor(
                            out=m2[:],
                            in0=G[:].rearrange("p s (h c) -> p s h c", h=4),
                            in1=exB[:, :, :, :],
                            op=ALU.mult,
                        )
                        if t_mdump is not None and L == 1 and s == 0:
                            nc.sync.dma_start(
                                out=t_mdump[:, :],
                                in_=m2[:].rearrange("p s c -> p (s c)"))
                        accM = ppool.tile([128, HC], F32, tag="accM")
                        accE = ppool.tile([128, 4], F32, tag="accE")
                        for sg in range(NSG):
                            nc.tensor.matmul(out=accM[:],
                                             lhsT=S[:, sg, :],
                                             rhs=m2[:, sg, :], start=(sg == 0),
                                             stop=(sg == NSG - 1))
                            nc.tensor.matmul(out=accE[:],
                                             lhsT=S[:, sg, :],
                                             rhs=exB[:, sg, :, 0],
                                             start=(sg == 0), stop=(sg == NSG - 1))

                        if op_limit < 5:
                            hOut = spool.tile([128, HC], F16, tag="hOut")
                            nc.vector.tensor_copy(out=hOut[:], in_=accM[:])
                            h_sink(s, hOut, mval, pool, ppool)
                            continue
                        if t_acdump is not None and L == 1 and s == 0:
                            acs = spool.tile([128, HC + 4], F32, tag="acdbg")
                            nc.vector.tensor_copy(out=acs[:, 0:HC], in_=accM[:])
                            nc.vector.tensor_copy(out=acs[:, HC:HC + 4],
                                                  in_=accE[:])
                            nc.sync.dma_start(out=t_acdump[:, :], in_=acs[:])
                        # flush: h = relu(accM/den + bias)
                        den = spool.tile([128, 4], F32, tag="den")
                        nc.vector.tensor_scalar(
                            out=den[:], in0=accE[:], scalar1=1e-30,
                            scalar2=None, op0=ALU.add,
                        )
                        rden = spool.tile([128, 4], F32, tag="rden")
                        nc.vector.reciprocal(out=rden[:], in_=den[:])
                        hT = spool.tile([128, HC], F16, tag="hT")
                        for hh in range(4):
                            blks = slice(hh * C, (hh + 1) * C)
                            nc.vector.scalar_tensor_tensor(
                                out=hT[:, blks], in0=accM[:, blks],
                                scalar=rden[:, hh:hh + 1], in1=bias_rep[:, blks],
                                op0=ALU.mult, op1=ALU.add,
                            )
                        hOut = spool.tile([128, HC], F16, tag="hOut")
                        nc.scalar.activation(out=hOut[:], in_=hT[:], func=AF.Relu)
                        h_sink(s, hOut, mval, pool, ppool)

            # layer 1: sink writes h1 to DRAM
            def h1_sink(s, hOut, mval, pool, ppool):
                nc.sync.dma_start(out=t_h1[s * 128:(s + 1) * 128, :], in_=hOut[:])
                if t_h1o is not None:
                    nc.sync.dma_start(out=t_h1o[s * 128:(s + 1) * 128, :],
                                      in_=hOut[:])

            if phase_limit >= 2:
                gat_layer(1, t_xl1, t_xr1, h1_sink)

            # --------------------------------------------------------------
            # Phase 4: xl2 from h1 -> AllGather kickoff -> xr2
            # (xr2 compute overlaps the collective; layer 2's R-gathers are
            # the only consumers of xr2 and start after the table arrives)
            # --------------------------------------------------------------
            with tc.tile_pool(name="p4", bufs=3) as pool, \
                 tc.tile_pool(name="p4ps", bufs=2, space="PSUM") as ppool:
                def xw2(s, nm0, sink0, nm1, sink1):
                    h1T0 = pool.tile([128, 128], F16, tag="h1T0")
                    h1T1 = pool.tile([128, 128], F16, tag="h1T1")
                    nc.sync.dma_start(
                        out=h1T0[:], in_=t_h1[s * 128:(s + 1) * 128, 0:128],
                        transpose=True,
                    )
                    nc.sync.dma_start(
                        out=h1T1[:], in_=t_h1[s * 128:(s + 1) * 128, 128:256],
                        transpose=True,
                    )
                    for nm, sink, dt in ((nm0, sink0, F16), (nm1, sink1, F8)):
                        wa, wb, wc = w2_tiles[nm]
                        ps = ppool.tile([128, HC], F32, tag="ps")
                        nc.tensor.matmul(out=ps[:], lhsT=h1T0[:], rhs=wa[:],
                                         start=True, stop=False)
                        nc.tensor.matmul(out=ps[:], lhsT=h1T1[:], rhs=wb[:],
                                         start=False, stop=False)
                        nc.tensor.matmul(out=ps[:], lhsT=ones_col[:],
                                         rhs=wc[:], start=False, stop=True)
                        xs = pool.tile([128, HC], dt, tag=f"xs{mybir.dt.size(dt)}")
                        nc.vector.tensor_copy(out=xs[:], in_=ps[:])
                        nc.sync.dma_start(out=sink[s * 128:(s + 1) * 128, :],
                                          in_=xs[:])

                for s in range(nspans if phase_limit >= 3 else 0):
                    xw2(s, "wl_aug", t_xl2_in, "wr_aug", t_xr2)

                # ----------------------------------------------------------
                # Phase 5: AllGather xl2
                # ----------------------------------------------------------
                if phase_limit >= 4:
                    nc.gpsimd.collective_compute(
                        "AllGather",
                        ALU.bypass,
                        replica_groups=[list(range(NCORES))],
                        ins=[t_xl2_in.ap().opt()],
                        outs=[t_xl2.ap().opt()],
                    )

            # --------------------------------------------------------------
            # Phase 6: GAT layer 2 with fused pooling
            # --------------------------------------------------------------
            if phase_limit < 5:
                with tc.tile_pool(name="dummyout", bufs=1) as dpool:
                    dz = dpool.tile([4, 64], F32)
                    nc.vector.memset(dz[:], 0.0)
                    nc.sync.dma_start(out=t_out[:], in_=dz[:])
                return
            with tc.tile_pool(name="gpool_ps", bufs=1, space="PSUM") as gpool_ps:
                gpsum = gpool_ps.tile([4, HC], F32)

                def h2_sink(s, hOut, mval, pool, ppool):
                    nc.tensor.matmul(out=gpsum[:], lhsT=mval[:, MV_G0:MV_G1],
                                     rhs=hOut[:],
                                     start=(s == 0), stop=(s == nspans - 1))

                gat_layer(2, t_xl2, t_xr2, h2_sink)

                # ----------------------------------------------------------
                # Phase 7: pooling -> MLP -> out
                # ----------------------------------------------------------
                with tc.tile_pool(name="mlp", bufs=1) as pool, \
                     tc.tile_pool(name="mlp_ps", bufs=2, space="PSUM") as ppool:
                    g = pool.tile([4, HC], F32)
                    nc.vector.tensor_scalar(out=g[:], in0=gpsum[:],
                                            scalar1=icnt[:, 0:1],
                                            scalar2=None, op0=ALU.mult)
                    # gT via PE transpose (two halves)
                    gT = pool.tile([128, 8], F32)
                    for half in range(2):
                        tp = ppool.tile([128, 128], F32, tag="tp")
                        nc.tensor.transpose(
                            out=tp[:, 0:4],
                            in_=g[:, half * 128:(half + 1) * 128],
                            identity=ident[0:4, 0:4],
                        )
                        nc.vector.tensor_copy(out=gT[:, half * 4:half * 4 + 4],
                                              in_=tp[:, 0:4])
                    z1p = ppool.tile([4, 128], F32, tag="z1p")
                    nc.tensor.matmul(out=z1p[:], lhsT=gT[:, 0:4], rhs=p1a[:],
                                     start=True, stop=False)
                    nc.tensor.matmul(out=z1p[:], lhsT=gT[:, 4:8], rhs=p1b[:],
                                     start=False, stop=False)
                    nc.tensor.matmul(out=z1p[:], lhsT=onesg[:], rhs=p1c[:],
                                     start=False, stop=True)
                    z1 = pool.tile([4, 128], F32)
                    nc.vector.tensor_copy(out=z1[:], in_=z1p[:])
                    # layernorm over free dim (128)
                    mu = pool.tile([4, 1], F32)
                    nc.vector.reduce_sum(out=mu[:], in_=z1[:], axis=AXX)
                    nc.vector.tensor_scalar(out=mu[:], in0=mu[:],
                                            scalar1=1.0 / 128,
                                            scalar2=None, op0=ALU.mult)
                    zc = pool.tile([4, 128], F32)
                    nc.vector.tensor_scalar(out=zc[:], in0=z1[:],
                                            scalar1=mu[:, 0:1],
                                            scalar2=None, op0=ALU.subtract)
                    sq = pool.tile([4, 128], F32)
                    nc.vector.tensor_tensor(out=sq[:], in0=zc[:], in1=zc[:],
                                            op=ALU.mult)
                    var = pool.tile([4, 1], F32)
                    nc.vector.reduce_sum(out=var[:], in_=sq[:], axis=AXX)
                    nc.vector.tensor_scalar(out=var[:], in0=var[:],
                                            scalar1=1.0 / 128,
                                            scalar2=1e-5, op0=ALU.mult,
                                            op1=ALU.add)
                    std = pool.tile([4, 1], F32)
                    nc.scalar.activation(out=std[:], in_=var[:], func=AF.Sqrt)
                    rstd = pool.tile([4, 1], F32)
                    nc.vector.reciprocal(out=rstd[:], in_=std[:])
                    zn = pool.tile([4, 128], F32)
                    nc.vector.tensor_scalar(out=zn[:], in0=zc[:],
                                            scalar1=rstd[:, 0:1],
                                            scalar2=None, op0=ALU.mult)
                    nc.vector.tensor_tensor(out=zn[:], in0=zn[:], in1=lng[:],
                                            op=ALU.mult)
                    nc.vector.tensor_tensor(out=zn[:], in0=zn[:], in1=lnb[:],
                                            op=ALU.add)
                    nc.scalar.activation(out=zn[:], in_=zn[:], func=AF.Relu)
                    # z2 = relu(zn @ p2 + b2)
                    znT = pool.tile([128, 4], F32)
                    tp2 = ppool.tile([128, 128], F32, tag="tp")
                    nc.tensor.transpose(out=tp2[:, 0:4], in_=zn[:],
                                        identity=ident[0:4, 0:4])
                    nc.vector.tensor_copy(out=znT[:], in_=tp2[:, 0:4])
                    z2p = ppool.tile([4, 64], F32, tag="z2p")
                    nc.tensor.matmul(out=z2p[:], lhsT=znT[:], rhs=p2a[:],
                                     start=True, stop=False)
                    nc.tensor.matmul(out=z2p[:], lhsT=onesg[:], rhs=p2c[:],
                                     start=False, stop=True)
                    zout = pool.tile([4, 64], F32)
                    nc.scalar.activation(out=zout[:], in_=z2p[:], func=AF.Relu)
                    nc.sync.dma_start(out=t_out[:], in_=zout[:])

        for _rep in range(reps):
            build_rep()

    nc.finalize()
    # Tile assigns SWDGE completion-sem lanes (8) round-robin over Pool DMA
    # insts in SCHEDULED order, and each lane must stick to one queue.  The
    # scheduler reorders gathers across spans, so rewrite queue_num in
    # scheduled order: lane i%8 <-> queue i%nqueues (nqueues | 8).
    i = 0
    for blk in nc.m.functions[0].blocks:
        for inst in blk.instructions:
            if inst.engine == mybir.EngineType.Pool and isinstance(
                inst, mybir.InstDMAGatherAnt
            ):
                inst.queue_num = i % nqueues
                i += 1
    return nc


# ----------------------------------------------------------------------------
# Entry point
# ----------------------------------------------------------------------------

def _pack_inputs(inp, cores, packs, nspans, rows_per_core, rows_total, x_aug_T):
    f16 = np.float16
    f32 = np.float32
    # shared (replicated) blocks
    wpack = np.zeros((WP_ROWS, HC), dtype=f16)

    def aug(w, b):
        return np.concatenate(
            [np.asarray(w, f32), np.asarray(b, f32)[None, :]], 0
        ).astype(f16)

    wpack[WP_WL1:WP_WL1 + 65] = aug(inp["g1_wl"], inp["g1_bl"])
    wpack[WP_WR1:WP_WR1 + 65] = aug(inp["g1_wr"], inp["g1_br"])
    wpack[WP_WL2:WP_WL2 + 257] = aug(inp["g2_wl"], inp["g2_bl"])
    wpack[WP_WR2:WP_WR2 + 257] = aug(inp["g2_wr"], inp["g2_br"])
    for L, (r_att, r_we, r_bias) in (
        (1, (WP_ATT1, WP_WE1, WP_BIAS1)),
        (2, (WP_ATT2, WP_WE2, WP_BIAS2)),
    ):
        wpack[r_att:r_att + 128] = np.broadcast_to(
            np.asarray(inp[f"g{L}_att"], f32).reshape(1, HC), (128, HC)
        ).astype(f16)
        wpack[r_we:r_we + 128] = np.broadcast_to(
            np.asarray(inp[f"g{L}_we"], f32).reshape(1, HC), (128, HC)
        ).astype(f16)
        wpack[r_bias:r_bias + 128] = np.broadcast_to(
            np.asarray(inp[f"g{L}_bias"], f32).reshape(1, HC), (128, HC)
        ).astype(f16)
    wpack[WP_IOTA:WP_IOTA + 128, 0:128] = np.broadcast_to(
        np.arange(128, dtype=f16)[None, :], (128, 128)
    )

    mlp = np.zeros((MP_ROWS, 128), dtype=f32)
    mlp[MP_P1:MP_P1 + 257] = np.concatenate(
        [np.asarray(inp["p1_w"], f32), np.asarray(inp["p1_b"], f32)[None, :]], 0
    )
    mlp[MP_LNG:MP_LNG + 4] = np.asarray(inp["ln_g"], f32)[None, :]
    mlp[MP_LNB:MP_LNB + 4] = np.asarray(inp["ln_b"], f32)[None, :]
    mlp[MP_P2:MP_P2 + 129, 0:64] = np.concatenate(
        [np.asarray(inp["p2_w"], f32), np.asarray(inp["p2_b"], f32)[None, :]], 0
    )

    enc_aug = np.concatenate(
        [np.asarray(inp["enc_w"], f32), np.asarray(inp["enc_b"], f32)[None, :]], 0
    )  # [5, 64]

    in_maps = []
    for k in range(NCORES):
        p = packs[k]
        feat = np.zeros((5, rows_total + rows_per_core + 64), dtype=f32)
        feat[:, 0:rows_total] = x_aug_T
        feat[:, rows_total:rows_total + rows_per_core] = x_aug_T[:, p["own_cols"]]
        feat[:, rows_total + rows_per_core:] = enc_aug
        mlp_k = mlp.copy()
        mlp_k[MP_CNT:MP_CNT + 4, 0] = p["inv_cnt"]
        in_maps.append({
            "feat": feat,
            "wpack": wpack,
            "meta_idx": p["meta_idx"],
            "meta_val": p["meta_val"].view(np.float16),
            "eaT": p["ea_T"],
            "mlp": mlp_k,
        })
    return in_maps


def kernel(**inputs):
    cores, packs, nspans, rows_per_core, rows_total, x_aug_T, node_row = _host_prep(
        inputs
    )
    key = (nspans, rows_total)
    if key not in _PROGRAM_CACHE:
        _PROGRAM_CACHE[key] = _build_program(nspans, rows_total)
    nc = _PROGRAM_CACHE[key]
    in_maps = _pack_inputs(
        inputs, cores, packs, nspans, rows_per_core, rows_total, x_aug_T
    )
    res = run_bass_kernel_spmd(nc, in_maps, core_ids=list(range(NCORES)))
    out = np.concatenate([res.results[k]["out"] for k in range(NCORES)], axis=0)
    return out.astype(np.float32)


if __name__ == "__main__":
    data = dict(np.load("/root/problem/inputs_cache.npz"))
    out = kernel(**data)
    exp = np.load("/root/problem/expected_np.npy")
    rel = np.linalg.norm(out - exp) / np.linalg.norm(exp)
    print("rel err:", rel)

